# revision 16
# baseline (speedup 1.0000x reference)
"""Trainium2 Bass kernel for paged-attention Llama-style block (nn_L4maAttention).

Sharding: tensor-parallel over heads across 8 NeuronCores. Core c owns
q-heads [4c, 4c+4), kv-head c, wq/wk/wv row shards and the matching wo
column shard. Each core computes a full [T, HID] partial of the output
projection in bf16; the host sums the 8 partials (the TP reduce).

Device kernel (per core), matmuls in bf16 (full PE rate, half DMA):
  phase 1a: K/V projections in 8 token chunks of 512, PSUM
            double-buffered (psk + psv + transpose bank) x 2 = 6 banks.
            K^T + RoPE -> KT (SBUF resident); V head-dim-major then
            PE-transposed to token-major tiles in V (SBUF resident).
            wq for phase 1b is prefetched here.
  phase 1b: Q projections in 8 chunks of 512, 4 heads x [128,512] PSUM
            (4 banks x 2 = all 8), RoPE -> QT (SBUF resident). RoPE
            half-swap via two partition-shifted SBUF->SBUF DMAs.
  phase 2+3 interleaved per batch b: causal attention for b's 8 (g,it)
            tiles, then b's slice of the output projection, so the
            projection's dense matmul stream overlaps the next batch's
            softmax scalar/vector work. Attention: transposed scores
            [k on partitions, q free] on the causal suffix of each
            128-row k block; exp in bf16 with 1/sqrt(d) folded into the
            activation scale; single [128,128] triangular mask on the
            diagonal sub-block; PV matmuls trail the score matmuls by
            one block so the PE never waits on the exp chain;
            denominators via a ones-matmul (broadcasts over partitions)
            + fast reciprocal.
"""

import math
import sys
from contextlib import ExitStack

import numpy as np

for _p in ("/opt/trn_rl_repo",):
    if _p not in sys.path:
        sys.path.insert(0, _p)

import concourse.mybir as mybir  # noqa: E402
import concourse.tile as tile  # noqa: E402
from concourse import bacc  # noqa: E402
from concourse.bass_utils import run_bass_kernel_spmd  # noqa: E402

NCORES = 8
HID = 4096
NH = 32
NKV = 8
HD = 128
B = 4
S = 1024
T = B * S
GQ = NH // NCORES          # q heads per core = 4
DQ = GQ * HD               # 512
KT32 = HID // 128          # 32 k tiles
QCH = 512                  # token chunk
NQCH = T // QCH            # 8
SCALE = 1.0 / math.sqrt(HD)

FP32 = mybir.dt.float32
BF16 = mybir.dt.bfloat16
NP_BF16 = mybir.dt.np(BF16)

_PROG_CACHE: dict = {}


def _llama31_freqs_np(head_dim: int) -> np.ndarray:
    half = head_dim // 2
    theta, scale, low_ff, high_ff, old_ctx = 500000.0, 8.0, 1.0, 4.0, 8192.0
    freq = 1.0 / (theta ** (np.arange(half, dtype=np.float64) * 2.0 / head_dim))
    wavelen = 2.0 * np.pi / freq
    low_wl, high_wl = old_ctx / low_ff, old_ctx / high_ff
    smooth = (old_ctx / wavelen - low_ff) / (high_ff - low_ff)
    out = np.where(
        wavelen < high_wl,
        freq,
        np.where(wavelen > low_wl, freq / scale, (1.0 - smooth) * freq / scale + smooth * freq),
    )
    return out.astype(np.float64)


def _rope_tables(pos: np.ndarray) -> tuple[np.ndarray, np.ndarray]:
    """cosF [128, n]: cos duplicated on both partition halves.
    sinF2 [128, n]: +sin on rows 0-63, -sin on rows 64-127. The kernel
    computes out = x*cosF + halfswap(x*sinF2), which equals rotate-half
    RoPE."""
    freqs = _llama31_freqs_np(HD)
    ang = pos.astype(np.float64)[None, :] * freqs[:, None]  # [64, n]
    c = np.cos(ang).astype(np.float32)
    s = np.sin(ang).astype(np.float32)
    cosF = np.concatenate([c, c], axis=0)
    sinF2 = np.concatenate([s, -s], axis=0)
    return np.ascontiguousarray(cosF), np.ascontiguousarray(sinF2)


def _build_program(split_kv: bool):
    nc = bacc.Bacc(
        "TRN2",
        target_bir_lowering=False,
        debug=False,
        enable_asserts=False,
        num_devices=NCORES,
    )
    hT = nc.dram_tensor("hT", [HID, T], BF16, kind="ExternalInput")
    hTkv = (
        nc.dram_tensor("hTkv", [HID, T], BF16, kind="ExternalInput") if split_kv else hT
    )
    wqT = nc.dram_tensor("wqT", [HID, DQ], BF16, kind="ExternalInput")
    wkT = nc.dram_tensor("wkT", [HID, HD], BF16, kind="ExternalInput")
    wvT = nc.dram_tensor("wvT", [HID, HD], BF16, kind="ExternalInput")
    woT = nc.dram_tensor("woT", [DQ, HID], BF16, kind="ExternalInput")
    # K (page-rank order) positions are always 0..S-1 per sequence; a
    # [128, S] table sliced modulo S covers both passes in the identity
    # case and the KV pass in the permuted case.
    coskv = nc.dram_tensor("coskv", [128, S], FP32, kind="ExternalInput")
    sinkv = nc.dram_tensor("sinkv", [128, S], FP32, kind="ExternalInput")
    if split_kv:
        cosq = nc.dram_tensor("cosq", [128, T], FP32, kind="ExternalInput")
        sinq = nc.dram_tensor("sinq", [128, T], FP32, kind="ExternalInput")
    trid = nc.dram_tensor("trid", [128, 128], BF16, kind="ExternalInput")
    onesd = nc.dram_tensor("onesd", [128, 128], BF16, kind="ExternalInput")
    outp = nc.dram_tensor("outp", [T, HID], BF16, kind="ExternalOutput")

    with tile.TileContext(nc) as tc, ExitStack() as ctx:
        const_pool = ctx.enter_context(tc.tile_pool(name="const", bufs=1))
        QT = const_pool.tile([128, GQ, T], BF16)        # 32KB/part
        KT = const_pool.tile([128, T], BF16)            # 8KB
        V = const_pool.tile([128, T // 128, HD], BF16)  # 8KB (token-major tiles)
        aoT = const_pool.tile([128, GQ, T], BF16)       # 32KB
        tri_sb = const_pool.tile([128, 128], BF16)
        ones_sb = const_pool.tile([128, 128], BF16)
        coskv_sb = const_pool.tile([128, S], FP32)
        sinkv_sb = const_pool.tile([128, S], FP32)
        # All DMAs stay on the single sync HWDGE queue: Tile assigns DMA
        # completion-sem lanes round-robin across queues while FIFO order
        # only holds per queue, so cross-queue DMAs can satisfy a
        # consumer's lane-wait out of order (observed as a flaky stale
        # weight read). Latency is managed by emission order instead:
        # constants are emitted inside the first chunk, below.
        if split_kv:
            cosq_sb = const_pool.tile([128, T], FP32)
            sinq_sb = const_pool.tile([128, T], FP32)
        else:
            cosq_sb, sinq_sb = coskv_sb, sinkv_sb

        def emit_const_dmas():
            nc.sync.dma_start(tri_sb[:], trid.ap()[:, :])
            nc.sync.dma_start(ones_sb[:], onesd.ap()[:, :])
            nc.sync.dma_start(coskv_sb[:], coskv.ap()[:, :])
            nc.sync.dma_start(sinkv_sb[:], sinkv.ap()[:, :])
            if split_kv:
                nc.sync.dma_start(cosq_sb[:], cosq.ap()[:, :])
                nc.sync.dma_start(sinq_sb[:], sinq.ap()[:, :])

        def rope_out(ps, cos_a, sin_a, out_full, tpool, n):
            """out = ps*cos + halfswap(ps*sinF2). The half-swap is two
            partition-shifted SBUF->SBUF DMAs (TensorTensor operands must
            share a start partition)."""
            t1 = tpool.tile([128, n], FP32, tag="t1", name="t1")
            u = tpool.tile([128, n], FP32, tag="u", name="u")
            u2 = tpool.tile([128, n], FP32, tag="u2", name="u2")
            nc.vector.tensor_mul(t1[:], ps[:], cos_a)
            nc.vector.tensor_mul(u[:], ps[:], sin_a)
            nc.sync.dma_start(u2[0:64, :], u[64:128, :])
            nc.sync.dma_start(u2[64:128, :], u[0:64, :])
            nc.vector.tensor_add(out_full, t1[:], u2[:])

        # Q pass runs FIRST (its 4-rope chunk evacuation is the expensive
        # one; the KV pass's cheaper evacuation then gates phase 2's PSUM
        # handoff). The h pool is shared by both passes so the second
        # pass's h DMAs start while the first pass is still computing.
        h_r = hT.ap().rearrange("(ko p) t -> p ko t", p=128)
        hkv_r = hTkv.ap().rearrange("(ko p) t -> p ko t", p=128)
        wq_r = wqT.ap().rearrange("(ko p) d -> p ko d", p=128)
        wk_r = wkT.ap().rearrange("(ko p) d -> p ko d", p=128)
        wv_r = wvT.ap().rearrange("(ko p) d -> p ko d", p=128)
        with tc.tile_pool(name="wqkv", bufs=1) as wpool, \
             tc.tile_pool(name="h1", bufs=5) as hpool:
            wq_sb = wpool.tile([128, KT32, DQ], BF16)
            wk_sb = wpool.tile([128, KT32, HD], BF16)
            wv_sb = wpool.tile([128, KT32, HD], BF16)
            # Startup streaming: wq k-groups are interleaved with chunk
            # 0's h tiles in sync-queue FIFO order, so the first matmuls
            # start after ~2MB instead of after all weights. Constants
            # land after chunk 0's h stream (needed at the first rope),
            # wk/wv after chunk 1's (needed a whole pass later).
            nc.sync.dma_start(wq_sb[:, 0:4, :], wq_r[:, 0:4, :])

            # ------------- phase 1b: Q projections -----------------------
            with tc.tile_pool(name="ps1b", bufs=2, space="PSUM") as ppool, \
                 tc.tile_pool(name="st1b", bufs=2) as stpool:
                for c in range(NQCH):
                    tsl = slice(c * QCH, (c + 1) * QCH)
                    hts = []
                    for j in range(4):
                        ht = hpool.tile([128, 8, QCH], BF16, tag="h", name="h")
                        nc.sync.dma_start(ht[:], h_r[:, j * 8:(j + 1) * 8, tsl])
                        hts.append(ht)
                        if c == 0:
                            for kq in (2 * j + 1, 2 * j + 2):
                                if kq < 8:
                                    nc.sync.dma_start(
                                        wq_sb[:, 4 * kq:4 * kq + 4, :],
                                        wq_r[:, 4 * kq:4 * kq + 4, :],
                                    )
                    if c == 0:
                        emit_const_dmas()
                    elif c == 1:
                        for kg in range(0, KT32, 8):
                            nc.sync.dma_start(
                                wk_sb[:, kg:kg + 8, :], wk_r[:, kg:kg + 8, :]
                            )
                            nc.sync.dma_start(
                                wv_sb[:, kg:kg + 8, :], wv_r[:, kg:kg + 8, :]
                            )
                    psq = [
                        ppool.tile([128, QCH], FP32, tag=f"psq{g}", name=f"psq{g}")
                        for g in range(GQ)
                    ]
                    for k in range(KT32):
                        ht = hts[k // 8][:, k % 8, :]
                        st = k == 0
                        sp = k == KT32 - 1
                        for g in range(GQ):
                            nc.tensor.matmul(
                                psq[g][:], wq_sb[:, k, g * 128:(g + 1) * 128], ht,
                                start=st, stop=sp,
                            )
                    if split_kv:
                        cs, ss = cosq_sb[:, tsl], sinq_sb[:, tsl]
                    else:
                        p0 = (c * QCH) % S
                        cs, ss = cosq_sb[:, p0:p0 + QCH], sinq_sb[:, p0:p0 + QCH]
                    for g in range(GQ):
                        rope_out(psq[g], cs, ss, QT[:, g, tsl], stpool, QCH)

            # ------------- phase 1a: K/V projections ---------------------
            with tc.tile_pool(name="ps1a", bufs=2, space="PSUM") as ppool, \
                 tc.tile_pool(name="st1a", bufs=2) as stpool:
                for c in range(NQCH):
                    tsl = slice(c * QCH, (c + 1) * QCH)
                    hts = []
                    for j in range(4):
                        ht = hpool.tile([128, 8, QCH], BF16, tag="h", name="h")
                        nc.sync.dma_start(ht[:], hkv_r[:, j * 8:(j + 1) * 8, tsl])
                        hts.append(ht)
                    psk = ppool.tile([128, QCH], FP32, tag="psk", name="psk")
                    psv = ppool.tile([128, QCH], FP32, tag="psv", name="psv")
                    for k in range(KT32):
                        ht = hts[k // 8][:, k % 8, :]
                        st = k == 0
                        sp = k == KT32 - 1
                        nc.tensor.matmul(psk[:], wk_sb[:, k, :], ht, start=st, stop=sp)
                        nc.tensor.matmul(psv[:], wv_sb[:, k, :], ht, start=st, stop=sp)
                    p0 = (c * QCH) % S
                    rope_out(
                        psk, coskv_sb[:, p0:p0 + QCH], sinkv_sb[:, p0:p0 + QCH],
                        KT[:, tsl], stpool, QCH,
                    )
                    vsb = stpool.tile([128, QCH], BF16, tag="vsb", name="vsb")
                    nc.scalar.copy(vsb[:], psv[:])
                    for i in range(4):
                        nc.sync.dma_start(
                            V[:, 4 * c + i, :], vsb[:, i * 128:(i + 1) * 128],
                            transpose=True,
                        )

        # ------------- phases 2+3, interleaved per batch ------------------
        wo_r = woT.ap().rearrange("(g p) e -> p g e", p=128)
        with tc.tile_pool(name="wo", bufs=1) as wopool:
            wo_sb = wopool.tile([128, GQ, HID], BF16)
            for g in range(GQ):
                nc.sync.dma_start(wo_sb[:, g, :], wo_r[:, g, :])

            with tc.tile_pool(name="sb2", bufs=2) as sbpool, \
                 tc.tile_pool(name="ex2", bufs=10) as expool, \
                 tc.tile_pool(name="pss2", bufs=2, space="PSUM") as pspool, \
                 tc.tile_pool(name="pv2", bufs=2, space="PSUM") as pvpool, \
                 tc.tile_pool(name="psd2", bufs=2, space="PSUM") as pdpool, \
                 tc.tile_pool(name="ps3", bufs=2, space="PSUM") as p3pool, \
                 tc.tile_pool(name="ob3", bufs=3) as obpool:
                for b in range(B):
                    # ---- phase 2 for batch b ----
                    for g in range(GQ):
                        for it in range(2):
                            qoff = it * QCH
                            q0 = b * S + qoff
                            njt = (qoff + QCH) // 128
                            es = sbpool.tile([128, QCH], BF16, tag="es", name="es")
                            pv = pvpool.tile([128, QCH], FP32, tag="pv", name="pv")
                            exs, offs = [], []
                            # All score matmuls stream first (the scalar
                            # engine's exps run one behind), then all PV
                            # matmuls — the PE never waits on the exp chain.
                            for jt in range(njt):
                                ko = b * S + jt * 128
                                off = jt * 128 - qoff if jt * 128 >= qoff else 0
                                pss = pspool.tile(
                                    [128, QCH], FP32, tag="pss", name="pss"
                                )
                                nc.tensor.matmul(
                                    pss[:, off:QCH],
                                    KT[:, ko:ko + 128],
                                    QT[:, g, q0 + off:q0 + QCH],
                                    start=True, stop=True,
                                )
                                ex = expool.tile([128, QCH], BF16, tag="ex", name="ex")
                                nc.scalar.activation(
                                    ex[:, off:QCH], pss[:, off:QCH],
                                    mybir.ActivationFunctionType.Exp, scale=SCALE,
                                )
                                if jt * 128 >= qoff:  # diagonal block
                                    nc.vector.tensor_mul(
                                        ex[:, off:off + 128], ex[:, off:off + 128],
                                        tri_sb[:],
                                    )
                                if jt == 0:
                                    nc.vector.tensor_copy(es[:], ex[:])
                                else:
                                    nc.vector.tensor_add(
                                        es[:, off:QCH], es[:, off:QCH], ex[:, off:QCH]
                                    )
                                exs.append(ex)
                                offs.append(off)
                            for jt in range(njt):
                                nc.tensor.matmul(
                                    pv[:, offs[jt]:QCH],
                                    V[:, b * 8 + jt, :],
                                    exs[jt][:, offs[jt]:QCH],
                                    start=(jt == 0), stop=(jt == njt - 1),
                                )
                            psd = pdpool.tile([128, QCH], FP32, tag="psd", name="psd")
                            nc.tensor.matmul(
                                psd[:], ones_sb[:], es[:], start=True, stop=True
                            )
                            rec = sbpool.tile([128, QCH], FP32, tag="rec", name="rec")
                            nc.vector.reciprocal_approx_fast(rec[:], psd[:])
                            nc.vector.tensor_mul(
                                aoT[:, g, q0:q0 + QCH], pv[:], rec[:]
                            )
                    # ---- phase 3 for batch b's tokens ----
                    for eg in range(2):
                        for tb in range(8 * b, 8 * b + 8):
                            ob = obpool.tile([128, 4, QCH], BF16, tag="ob", name="ob")
                            for ei in range(4):
                                e0 = eg * 2048 + ei * QCH
                                pso = p3pool.tile(
                                    [128, QCH], FP32, tag="pso", name="pso"
                                )
                                for g in range(GQ):
                                    nc.tensor.matmul(
                                        pso[:],
                                        aoT[:, g, tb * 128:(tb + 1) * 128],
                                        wo_sb[:, g, e0:e0 + QCH],
                                        start=(g == 0), stop=(g == GQ - 1),
                                    )
                                if ei % 2 == 0:
                                    nc.scalar.copy(ob[:, ei, :], pso[:])
                                else:
                                    nc.vector.tensor_copy(ob[:, ei, :], pso[:])
                            nc.sync.dma_start(
                                outp.ap()[tb * 128:(tb + 1) * 128,
                                          eg * 2048:(eg + 1) * 2048],
                                ob[:],
                            )

    nc.finalize()
    return nc


def _get_program(split_kv: bool):
    if split_kv not in _PROG_CACHE:
        _PROG_CACHE[split_kv] = _build_program(split_kv)
    return _PROG_CACHE[split_kv]


def kernel(
    hidden_states, wq, wk, wv, wo, kv_cache, position_ids,
    kv_page_indices, kv_page_indptr, kv_last_page_lens, qo_indptr,
    _run_kwargs: dict | None = None,
):
    hidden_states = np.asarray(hidden_states, np.float32)
    wq = np.asarray(wq, np.float32)
    wk = np.asarray(wk, np.float32)
    wv = np.asarray(wv, np.float32)
    wo = np.asarray(wo, np.float32)
    position_ids = np.asarray(position_ids, np.int32)
    qo_indptr = np.asarray(qo_indptr, np.int64)

    nnz = hidden_states.shape[0]
    b = qo_indptr.shape[0] - 1
    assert nnz == T and b == B, (nnz, b)
    assert np.array_equal(qo_indptr, np.arange(B + 1, dtype=np.int64) * S), (
        "kernel assumes uniform sequence lengths of 1024"
    )

    # Page-gather order: the reference gathers pages in list order, so the
    # token with position p within its sequence lands at page-order rank p.
    # KV must be fed in rank order; the q path stays in token order.
    perm = np.empty(T, np.int64)
    identity = True
    for bi in range(B):
        pos_b = position_ids[bi * S:(bi + 1) * S].astype(np.int64)
        assert np.array_equal(np.sort(pos_b), np.arange(S)), (
            "kernel assumes positions cover 0..S-1 exactly once per sequence"
        )
        inv = np.empty(S, np.int64)
        inv[pos_b] = np.arange(S)
        perm[bi * S:(bi + 1) * S] = bi * S + inv
        if not np.array_equal(inv, np.arange(S)):
            identity = False

    hT16 = np.ascontiguousarray(hidden_states.T.astype(NP_BF16))
    coskv, sinkv = _rope_tables(np.arange(S, dtype=np.int64))
    tri = np.ascontiguousarray(
        (np.arange(128)[:, None] <= np.arange(128)[None, :]).astype(NP_BF16)
    )
    ones = np.ones((128, 128), NP_BF16)

    split_kv = not identity
    nc = _get_program(split_kv)

    in_maps = []
    for c in range(NCORES):
        im = {
            "hT": hT16,
            "wqT": np.ascontiguousarray(wq[c * DQ:(c + 1) * DQ, :].T.astype(NP_BF16)),
            "wkT": np.ascontiguousarray(wk[c * HD:(c + 1) * HD, :].T.astype(NP_BF16)),
            "wvT": np.ascontiguousarray(wv[c * HD:(c + 1) * HD, :].T.astype(NP_BF16)),
            "woT": np.ascontiguousarray(wo[:, c * DQ:(c + 1) * DQ].T.astype(NP_BF16)),
            "coskv": coskv,
            "sinkv": sinkv,
            "trid": tri,
            "onesd": ones,
        }
        if split_kv:
            im["hTkv"] = np.ascontiguousarray(hT16[:, perm])
            cosq, sinq = _rope_tables(position_ids)
            im["cosq"] = cosq
            im["sinq"] = sinq
        in_maps.append(im)

    res = run_bass_kernel_spmd(
        nc, in_maps, core_ids=list(range(NCORES)), **(_run_kwargs or {})
    )
    out = np.zeros((T, HID), np.float32)
    for c in range(NCORES):
        out += res.results[c]["outp"].astype(np.float32)
    kernel.last_results = res  # type: ignore[attr-defined]
    return out


# revision 24
# speedup vs baseline: 1.0990x; 1.0990x over previous
"""Trainium2 Bass kernel for paged-attention Llama-style block (nn_L4maAttention).

Sharding: tensor-parallel over heads across 8 NeuronCores. Core c owns
q-heads [4c, 4c+4), kv-head c, wq/wk/wv row shards and the matching wo
column shard. Each core computes a full [T, HID] partial of the output
projection in bf16; the host sums the 8 partials (the TP reduce).

Device kernel (per core), matmuls in bf16 (full PE rate, half DMA):
  phase 1a: K/V projections in 8 token chunks of 512, PSUM
            double-buffered (psk + psv + transpose bank) x 2 = 6 banks.
            K^T + RoPE -> KT (SBUF resident); V head-dim-major then
            PE-transposed to token-major tiles in V (SBUF resident).
            wq for phase 1b is prefetched here.
  phase 1b: Q projections in 8 chunks of 512, 4 heads x [128,512] PSUM
            (4 banks x 2 = all 8), RoPE -> QT (SBUF resident). RoPE
            half-swap via two partition-shifted SBUF->SBUF DMAs.
  phase 2+3 interleaved per batch b: causal attention for b's 8 (g,it)
            tiles, then b's slice of the output projection, so the
            projection's dense matmul stream overlaps the next batch's
            softmax scalar/vector work. Attention: transposed scores
            [k on partitions, q free] on the causal suffix of each
            128-row k block; exp in bf16 with 1/sqrt(d) folded into the
            activation scale; single [128,128] triangular mask on the
            diagonal sub-block; PV matmuls trail the score matmuls by
            one block so the PE never waits on the exp chain;
            denominators via a ones-matmul (broadcasts over partitions)
            + fast reciprocal.
"""

import math
import sys
from contextlib import ExitStack

import numpy as np

for _p in ("/opt/trn_rl_repo",):
    if _p not in sys.path:
        sys.path.insert(0, _p)

import concourse.mybir as mybir  # noqa: E402
import concourse.tile as tile  # noqa: E402
from concourse import bacc  # noqa: E402
from concourse.bass_utils import run_bass_kernel_spmd  # noqa: E402

NCORES = 8
HID = 4096
NH = 32
NKV = 8
HD = 128
B = 4
S = 1024
T = B * S
GQ = NH // NCORES          # q heads per core = 4
DQ = GQ * HD               # 512
KT32 = HID // 128          # 32 k tiles
QCH = 512                  # token chunk
NQCH = T // QCH            # 8
SCALE = 1.0 / math.sqrt(HD)

FP32 = mybir.dt.float32
BF16 = mybir.dt.bfloat16
NP_BF16 = mybir.dt.np(BF16)

_PROG_CACHE: dict = {}


def _llama31_freqs_np(head_dim: int) -> np.ndarray:
    half = head_dim // 2
    theta, scale, low_ff, high_ff, old_ctx = 500000.0, 8.0, 1.0, 4.0, 8192.0
    freq = 1.0 / (theta ** (np.arange(half, dtype=np.float64) * 2.0 / head_dim))
    wavelen = 2.0 * np.pi / freq
    low_wl, high_wl = old_ctx / low_ff, old_ctx / high_ff
    smooth = (old_ctx / wavelen - low_ff) / (high_ff - low_ff)
    out = np.where(
        wavelen < high_wl,
        freq,
        np.where(wavelen > low_wl, freq / scale, (1.0 - smooth) * freq / scale + smooth * freq),
    )
    return out.astype(np.float64)


def _rope_tables(pos: np.ndarray) -> tuple[np.ndarray, np.ndarray]:
    """cosF [128, n]: cos duplicated on both partition halves.
    sinF2 [128, n]: +sin on rows 0-63, -sin on rows 64-127. The kernel
    computes out = x*cosF + halfswap(x*sinF2), which equals rotate-half
    RoPE."""
    freqs = _llama31_freqs_np(HD)
    ang = pos.astype(np.float64)[None, :] * freqs[:, None]  # [64, n]
    c = np.cos(ang).astype(np.float32)
    s = np.sin(ang).astype(np.float32)
    cosF = np.concatenate([c, c], axis=0)
    sinF2 = np.concatenate([s, -s], axis=0)
    return np.ascontiguousarray(cosF), np.ascontiguousarray(sinF2)


def _build_program(split_kv: bool):
    nc = bacc.Bacc(
        "TRN2",
        target_bir_lowering=False,
        debug=False,
        enable_asserts=False,
        num_devices=NCORES,
    )
    hT = nc.dram_tensor("hT", [HID, T], BF16, kind="ExternalInput")
    hTkv = (
        nc.dram_tensor("hTkv", [HID, T], BF16, kind="ExternalInput") if split_kv else hT
    )
    wqT = nc.dram_tensor("wqT", [HID, DQ], BF16, kind="ExternalInput")
    wkT = nc.dram_tensor("wkT", [HID, HD], BF16, kind="ExternalInput")
    wvT = nc.dram_tensor("wvT", [HID, HD], BF16, kind="ExternalInput")
    woT = nc.dram_tensor("woT", [DQ, HID], BF16, kind="ExternalInput")
    # K (page-rank order) positions are always 0..S-1 per sequence; a
    # [128, S] table sliced modulo S covers both passes in the identity
    # case and the KV pass in the permuted case.
    coskv = nc.dram_tensor("coskv", [128, S], FP32, kind="ExternalInput")
    sinkv = nc.dram_tensor("sinkv", [128, S], FP32, kind="ExternalInput")
    if split_kv:
        cosq = nc.dram_tensor("cosq", [128, T], FP32, kind="ExternalInput")
        sinq = nc.dram_tensor("sinq", [128, T], FP32, kind="ExternalInput")
    trid = nc.dram_tensor("trid", [128, 128], BF16, kind="ExternalInput")
    onesd = nc.dram_tensor("onesd", [128, 128], BF16, kind="ExternalInput")
    identd = nc.dram_tensor("identd", [128, 128], BF16, kind="ExternalInput")
    outp = nc.dram_tensor("outp", [T, HID], BF16, kind="ExternalOutput")

    with tile.TileContext(nc) as tc, ExitStack() as ctx:
        const_pool = ctx.enter_context(tc.tile_pool(name="const", bufs=1))
        QT = const_pool.tile([128, GQ, T], BF16)        # 32KB/part
        KT = const_pool.tile([128, T], BF16)            # 8KB
        V = const_pool.tile([128, T // 128, HD], BF16)  # 8KB (token-major tiles)
        aoT = const_pool.tile([128, GQ, T], BF16)       # 32KB
        tri_sb = const_pool.tile([128, 128], BF16)
        ones_sb = const_pool.tile([128, 128], BF16)
        ident_sb = const_pool.tile([128, 128], BF16)
        coskv_sb = const_pool.tile([128, S], FP32)
        sinkv_sb = const_pool.tile([128, S], FP32)
        # All DMAs stay on the single sync HWDGE queue: Tile assigns DMA
        # completion-sem lanes round-robin across queues while FIFO order
        # only holds per queue, so cross-queue DMAs can satisfy a
        # consumer's lane-wait out of order (observed as a flaky stale
        # weight read). Latency is managed by emission order instead:
        # constants are emitted inside the first chunk, below.
        if split_kv:
            cosq_sb = const_pool.tile([128, T], FP32)
            sinq_sb = const_pool.tile([128, T], FP32)
        else:
            cosq_sb, sinq_sb = coskv_sb, sinkv_sb

        def emit_const_dmas():
            nc.sync.dma_start(tri_sb[:], trid.ap()[:, :])
            nc.sync.dma_start(ones_sb[:], onesd.ap()[:, :])
            nc.sync.dma_start(ident_sb[:], identd.ap()[:, :])
            nc.sync.dma_start(coskv_sb[:], coskv.ap()[:, :])
            nc.sync.dma_start(sinkv_sb[:], sinkv.ap()[:, :])
            if split_kv:
                nc.sync.dma_start(cosq_sb[:], cosq.ap()[:, :])
                nc.sync.dma_start(sinq_sb[:], sinq.ap()[:, :])

        def rope_out(ps, cos_a, sin_a, out_full, tpool, n):
            """out = ps*cos + halfswap(ps*sinF2). The half-swap is two
            partition-shifted SBUF->SBUF DMAs (TensorTensor operands must
            share a start partition)."""
            t1 = tpool.tile([128, n], FP32, tag="t1", name="t1")
            u = tpool.tile([128, n], FP32, tag="u", name="u")
            u2 = tpool.tile([128, n], FP32, tag="u2", name="u2")
            nc.vector.tensor_mul(t1[:], ps[:], cos_a)
            nc.vector.tensor_mul(u[:], ps[:], sin_a)
            nc.sync.dma_start(u2[0:64, :], u[64:128, :])
            nc.sync.dma_start(u2[64:128, :], u[0:64, :])
            nc.vector.tensor_add(out_full, t1[:], u2[:])

        # Q pass runs FIRST (its 4-rope chunk evacuation is the expensive
        # one; the KV pass's cheaper evacuation then gates phase 2's PSUM
        # handoff). The h pool is shared by both passes so the second
        # pass's h DMAs start while the first pass is still computing.
        h_r = hT.ap().rearrange("(ko p) t -> p ko t", p=128)
        hkv_r = hTkv.ap().rearrange("(ko p) t -> p ko t", p=128)
        wq_r = wqT.ap().rearrange("(ko p) d -> p ko d", p=128)
        wk_r = wkT.ap().rearrange("(ko p) d -> p ko d", p=128)
        wv_r = wvT.ap().rearrange("(ko p) d -> p ko d", p=128)
        with tc.tile_pool(name="wqkv", bufs=1) as wpool, \
             tc.tile_pool(name="h1", bufs=5 if not split_kv else 4) as hpool:
            wq_sb = wpool.tile([128, KT32, DQ], BF16)
            wk_sb = wpool.tile([128, KT32, HD], BF16)
            wv_sb = wpool.tile([128, KT32, HD], BF16)
            # Startup streaming: wq k-groups are interleaved with chunk
            # 0's h tiles in sync-queue FIFO order, so the first matmuls
            # start after ~2MB instead of after all weights. Constants
            # land after chunk 0's h stream (needed at the first rope),
            # wk/wv after chunk 1's (needed a whole pass later).
            nc.sync.dma_start(wq_sb[:, 0:4, :], wq_r[:, 0:4, :])

            # ------------- phase 1b: Q projections -----------------------
            with tc.tile_pool(name="ps1b", bufs=2, space="PSUM") as ppool, \
                 tc.tile_pool(name="st1b", bufs=2) as stpool:
                for c in range(NQCH):
                    tsl = slice(c * QCH, (c + 1) * QCH)
                    hts = []
                    for j in range(4):
                        ht = hpool.tile([128, 8, QCH], BF16, tag="h", name="h")
                        nc.sync.dma_start(ht[:], h_r[:, j * 8:(j + 1) * 8, tsl])
                        hts.append(ht)
                        if c == 0:
                            for kq in (2 * j + 1, 2 * j + 2):
                                if kq < 8:
                                    nc.sync.dma_start(
                                        wq_sb[:, 4 * kq:4 * kq + 4, :],
                                        wq_r[:, 4 * kq:4 * kq + 4, :],
                                    )
                    if c == 0:
                        emit_const_dmas()
                    elif c == 1:
                        for kg in range(0, KT32, 8):
                            nc.sync.dma_start(
                                wk_sb[:, kg:kg + 8, :], wk_r[:, kg:kg + 8, :]
                            )
                            nc.sync.dma_start(
                                wv_sb[:, kg:kg + 8, :], wv_r[:, kg:kg + 8, :]
                            )
                    if split_kv:
                        cs, ss = cosq_sb[:, tsl], sinq_sb[:, tsl]
                    else:
                        p0 = (c * QCH) % S
                        cs, ss = cosq_sb[:, p0:p0 + QCH], sinq_sb[:, p0:p0 + QCH]
                    # The last chunk runs as two 2-head sub-passes so its
                    # final rope evacuation overlaps the second sub-pass's
                    # matmuls instead of stalling the next phase's PSUM
                    # handoff.
                    groups = (
                        [tuple(range(GQ))] if c < NQCH - 1 else [(0, 1), (2, 3)]
                    )
                    for gs in groups:
                        psq = {
                            g: ppool.tile(
                                [128, QCH], FP32, tag=f"psq{g}", name=f"psq{g}"
                            )
                            for g in gs
                        }
                        for k in range(KT32):
                            ht = hts[k // 8][:, k % 8, :]
                            st = k == 0
                            sp = k == KT32 - 1
                            for g in gs:
                                nc.tensor.matmul(
                                    psq[g][:], wq_sb[:, k, g * 128:(g + 1) * 128],
                                    ht, start=st, stop=sp,
                                )
                        for g in gs:
                            rope_out(psq[g], cs, ss, QT[:, g, tsl], stpool, QCH)

            # ------------- phase 1a: K/V projections ---------------------
            with tc.tile_pool(name="ps1a", bufs=2, space="PSUM") as ppool, \
                 tc.tile_pool(name="st1a", bufs=2) as stpool:
                for c in range(NQCH):
                    tsl = slice(c * QCH, (c + 1) * QCH)
                    hts = []
                    for j in range(4):
                        ht = hpool.tile([128, 8, QCH], BF16, tag="h", name="h")
                        nc.sync.dma_start(ht[:], hkv_r[:, j * 8:(j + 1) * 8, tsl])
                        hts.append(ht)
                    p0 = (c * QCH) % S

                    def emit_k(psk):
                        rope_out(
                            psk, coskv_sb[:, p0:p0 + QCH], sinkv_sb[:, p0:p0 + QCH],
                            KT[:, tsl], stpool, QCH,
                        )

                    def emit_v(psv):
                        vsb = stpool.tile([128, QCH], BF16, tag="vsb", name="vsb")
                        nc.scalar.copy(vsb[:], psv[:])
                        pst = ppool.tile([128, 4, 128], BF16, tag="pst", name="pst")
                        for i in range(4):
                            nc.tensor.transpose(
                                pst[:, i, :], vsb[:, i * 128:(i + 1) * 128],
                                ident_sb[:],
                            )
                            nc.vector.tensor_copy(V[:, 4 * c + i, :], pst[:, i, :])

                    if c < NQCH - 1:
                        psk = ppool.tile([128, QCH], FP32, tag="psk", name="psk")
                        psv = ppool.tile([128, QCH], FP32, tag="psv", name="psv")
                        for k in range(KT32):
                            ht = hts[k // 8][:, k % 8, :]
                            st = k == 0
                            sp = k == KT32 - 1
                            nc.tensor.matmul(
                                psk[:], wk_sb[:, k, :], ht, start=st, stop=sp
                            )
                            nc.tensor.matmul(
                                psv[:], wv_sb[:, k, :], ht, start=st, stop=sp
                            )
                        emit_k(psk)
                        emit_v(psv)
                    else:
                        # Last chunk: K sweep then V sweep, so the K rope
                        # overlaps the V matmuls and only the short V
                        # evacuation gates the attention phase.
                        psk = ppool.tile([128, QCH], FP32, tag="psk", name="psk")
                        for k in range(KT32):
                            nc.tensor.matmul(
                                psk[:], wk_sb[:, k, :], hts[k // 8][:, k % 8, :],
                                start=(k == 0), stop=(k == KT32 - 1),
                            )
                        emit_k(psk)
                        psv = ppool.tile([128, QCH], FP32, tag="psv", name="psv")
                        for k in range(KT32):
                            nc.tensor.matmul(
                                psv[:], wv_sb[:, k, :], hts[k // 8][:, k % 8, :],
                                start=(k == 0), stop=(k == KT32 - 1),
                            )
                        emit_v(psv)

        # ------------- phases 2+3, interleaved per batch ------------------
        wo_r = woT.ap().rearrange("(g p) e -> p g e", p=128)
        with tc.tile_pool(name="wo", bufs=1) as wopool:
            wo_sb = wopool.tile([128, GQ, HID], BF16)
            for g in range(GQ):
                nc.sync.dma_start(wo_sb[:, g, :], wo_r[:, g, :])

            with tc.tile_pool(name="sb2", bufs=2) as sbpool, \
                 tc.tile_pool(name="ex2", bufs=10) as expool, \
                 tc.tile_pool(name="pss2", bufs=2, space="PSUM") as pspool, \
                 tc.tile_pool(name="pv2", bufs=2, space="PSUM") as pvpool, \
                 tc.tile_pool(name="psd2", bufs=2, space="PSUM") as pdpool, \
                 tc.tile_pool(name="ps3", bufs=2, space="PSUM") as p3pool, \
                 tc.tile_pool(name="ob3", bufs=3) as obpool:
                for b in range(B):
                    # ---- phase 2 for batch b ----
                    for g in range(GQ):
                        for it in range(2):
                            qoff = it * QCH
                            q0 = b * S + qoff
                            njt = (qoff + QCH) // 128
                            es = sbpool.tile([128, QCH], BF16, tag="es", name="es")
                            pv = pvpool.tile([128, QCH], FP32, tag="pv", name="pv")
                            exs, offs = [], []
                            # All score matmuls stream first (the scalar
                            # engine's exps run one behind), then all PV
                            # matmuls — the PE never waits on the exp chain.
                            for jt in range(njt):
                                ko = b * S + jt * 128
                                off = jt * 128 - qoff if jt * 128 >= qoff else 0
                                pss = pspool.tile(
                                    [128, QCH], FP32, tag="pss", name="pss"
                                )
                                nc.tensor.matmul(
                                    pss[:, off:QCH],
                                    KT[:, ko:ko + 128],
                                    QT[:, g, q0 + off:q0 + QCH],
                                    start=True, stop=True,
                                )
                                ex = expool.tile([128, QCH], BF16, tag="ex", name="ex")
                                nc.scalar.activation(
                                    ex[:, off:QCH], pss[:, off:QCH],
                                    mybir.ActivationFunctionType.Exp, scale=SCALE,
                                )
                                if jt * 128 >= qoff:  # diagonal block
                                    nc.vector.tensor_mul(
                                        ex[:, off:off + 128], ex[:, off:off + 128],
                                        tri_sb[:],
                                    )
                                if jt == 0:
                                    nc.vector.tensor_copy(es[:], ex[:])
                                else:
                                    nc.vector.tensor_add(
                                        es[:, off:QCH], es[:, off:QCH], ex[:, off:QCH]
                                    )
                                exs.append(ex)
                                offs.append(off)
                            for jt in range(njt):
                                nc.tensor.matmul(
                                    pv[:, offs[jt]:QCH],
                                    V[:, b * 8 + jt, :],
                                    exs[jt][:, offs[jt]:QCH],
                                    start=(jt == 0), stop=(jt == njt - 1),
                                )
                            psd = pdpool.tile([128, QCH], FP32, tag="psd", name="psd")
                            nc.tensor.matmul(
                                psd[:], ones_sb[:], es[:], start=True, stop=True
                            )
                            rec = sbpool.tile([128, QCH], FP32, tag="rec", name="rec")
                            nc.vector.reciprocal_approx_fast(rec[:], psd[:])
                            nc.vector.tensor_mul(
                                aoT[:, g, q0:q0 + QCH], pv[:], rec[:]
                            )
                    # ---- phase 3 for batch b's tokens ----
                    for eg in range(2):
                        for tb in range(8 * b, 8 * b + 8):
                            ob = obpool.tile([128, 4, QCH], BF16, tag="ob", name="ob")
                            for ei in range(4):
                                e0 = eg * 2048 + ei * QCH
                                pso = p3pool.tile(
                                    [128, QCH], FP32, tag="pso", name="pso"
                                )
                                for g in range(GQ):
                                    nc.tensor.matmul(
                                        pso[:],
                                        aoT[:, g, tb * 128:(tb + 1) * 128],
                                        wo_sb[:, g, e0:e0 + QCH],
                                        start=(g == 0), stop=(g == GQ - 1),
                                    )
                                if ei % 2 == 0:
                                    nc.scalar.copy(ob[:, ei, :], pso[:])
                                else:
                                    nc.vector.tensor_copy(ob[:, ei, :], pso[:])
                                if ei == 1:
                                    nc.sync.dma_start(
                                        outp.ap()[tb * 128:(tb + 1) * 128,
                                                  eg * 2048:eg * 2048 + 1024],
                                        ob[:, 0:2, :],
                                    )
                            nc.sync.dma_start(
                                outp.ap()[tb * 128:(tb + 1) * 128,
                                          eg * 2048 + 1024:(eg + 1) * 2048],
                                ob[:, 2:4, :],
                            )

    nc.finalize()
    return nc


def _get_program(split_kv: bool):
    if split_kv not in _PROG_CACHE:
        _PROG_CACHE[split_kv] = _build_program(split_kv)
    return _PROG_CACHE[split_kv]


def kernel(
    hidden_states, wq, wk, wv, wo, kv_cache, position_ids,
    kv_page_indices, kv_page_indptr, kv_last_page_lens, qo_indptr,
    _run_kwargs: dict | None = None,
):
    hidden_states = np.asarray(hidden_states, np.float32)
    wq = np.asarray(wq, np.float32)
    wk = np.asarray(wk, np.float32)
    wv = np.asarray(wv, np.float32)
    wo = np.asarray(wo, np.float32)
    position_ids = np.asarray(position_ids, np.int32)
    qo_indptr = np.asarray(qo_indptr, np.int64)

    nnz = hidden_states.shape[0]
    b = qo_indptr.shape[0] - 1
    assert nnz == T and b == B, (nnz, b)
    assert np.array_equal(qo_indptr, np.arange(B + 1, dtype=np.int64) * S), (
        "kernel assumes uniform sequence lengths of 1024"
    )

    # Page-gather order: the reference gathers pages in list order, so the
    # token with position p within its sequence lands at page-order rank p.
    # KV must be fed in rank order; the q path stays in token order.
    perm = np.empty(T, np.int64)
    identity = True
    for bi in range(B):
        pos_b = position_ids[bi * S:(bi + 1) * S].astype(np.int64)
        assert np.array_equal(np.sort(pos_b), np.arange(S)), (
            "kernel assumes positions cover 0..S-1 exactly once per sequence"
        )
        inv = np.empty(S, np.int64)
        inv[pos_b] = np.arange(S)
        perm[bi * S:(bi + 1) * S] = bi * S + inv
        if not np.array_equal(inv, np.arange(S)):
            identity = False

    hT16 = np.ascontiguousarray(hidden_states.T.astype(NP_BF16))
    coskv, sinkv = _rope_tables(np.arange(S, dtype=np.int64))
    tri = np.ascontiguousarray(
        (np.arange(128)[:, None] <= np.arange(128)[None, :]).astype(NP_BF16)
    )
    ones = np.ones((128, 128), NP_BF16)
    eye = np.eye(128, dtype=np.float32).astype(NP_BF16)

    split_kv = not identity
    nc = _get_program(split_kv)

    in_maps = []
    for c in range(NCORES):
        im = {
            "hT": hT16,
            "wqT": np.ascontiguousarray(wq[c * DQ:(c + 1) * DQ, :].T.astype(NP_BF16)),
            "wkT": np.ascontiguousarray(wk[c * HD:(c + 1) * HD, :].T.astype(NP_BF16)),
            "wvT": np.ascontiguousarray(wv[c * HD:(c + 1) * HD, :].T.astype(NP_BF16)),
            "woT": np.ascontiguousarray(wo[:, c * DQ:(c + 1) * DQ].T.astype(NP_BF16)),
            "coskv": coskv,
            "sinkv": sinkv,
            "trid": tri,
            "onesd": ones,
            "identd": eye,
        }
        if split_kv:
            im["hTkv"] = np.ascontiguousarray(hT16[:, perm])
            cosq, sinq = _rope_tables(position_ids)
            im["cosq"] = cosq
            im["sinq"] = sinq
        in_maps.append(im)

    res = run_bass_kernel_spmd(
        nc, in_maps, core_ids=list(range(NCORES)), **(_run_kwargs or {})
    )
    out = np.zeros((T, HID), np.float32)
    for c in range(NCORES):
        out += res.results[c]["outp"].astype(np.float32)
    kernel.last_results = res  # type: ignore[attr-defined]
    return out


# revision 27
# speedup vs baseline: 1.1280x; 1.0264x over previous
"""Trainium2 Bass kernel for paged-attention Llama-style block (nn_L4maAttention).

Sharding: tensor-parallel over heads across 8 NeuronCores. Core c owns
q-heads [4c, 4c+4), kv-head c, wq/wk/wv row shards and the matching wo
column shard. Each core computes a full [T, HID] partial of the output
projection in bf16; the host sums the 8 partials (the TP reduce).

Device kernel (per core), matmuls in bf16 (full PE rate, half DMA):
  phase 1a: K/V projections in 8 token chunks of 512, PSUM
            double-buffered (psk + psv + transpose bank) x 2 = 6 banks.
            K^T + RoPE -> KT (SBUF resident); V head-dim-major then
            PE-transposed to token-major tiles in V (SBUF resident).
            wq for phase 1b is prefetched here.
  phase 1b: Q projections in 8 chunks of 512, 4 heads x [128,512] PSUM
            (4 banks x 2 = all 8), RoPE -> QT (SBUF resident). RoPE
            half-swap via two partition-shifted SBUF->SBUF DMAs.
  phase 2+3 interleaved per batch b: causal attention for b's 8 (g,it)
            tiles, then b's slice of the output projection, so the
            projection's dense matmul stream overlaps the next batch's
            softmax scalar/vector work. Attention: transposed scores
            [k on partitions, q free] on the causal suffix of each
            128-row k block; exp in bf16 with 1/sqrt(d) folded into the
            activation scale; single [128,128] triangular mask on the
            diagonal sub-block; PV matmuls trail the score matmuls by
            one block so the PE never waits on the exp chain;
            denominators via a ones-matmul (broadcasts over partitions)
            + fast reciprocal.
"""

import math
import sys
from contextlib import ExitStack

import numpy as np

for _p in ("/opt/trn_rl_repo",):
    if _p not in sys.path:
        sys.path.insert(0, _p)

import concourse.mybir as mybir  # noqa: E402
import concourse.tile as tile  # noqa: E402
from concourse import bacc  # noqa: E402
from concourse.bass_utils import run_bass_kernel_spmd  # noqa: E402

NCORES = 8
HID = 4096
NH = 32
NKV = 8
HD = 128
B = 4
S = 1024
T = B * S
GQ = NH // NCORES          # q heads per core = 4
DQ = GQ * HD               # 512
KT32 = HID // 128          # 32 k tiles
QCH = 512                  # token chunk
NQCH = T // QCH            # 8
SCALE = 1.0 / math.sqrt(HD)

FP32 = mybir.dt.float32
BF16 = mybir.dt.bfloat16
NP_BF16 = mybir.dt.np(BF16)

_PROG_CACHE: dict = {}


def _llama31_freqs_np(head_dim: int) -> np.ndarray:
    half = head_dim // 2
    theta, scale, low_ff, high_ff, old_ctx = 500000.0, 8.0, 1.0, 4.0, 8192.0
    freq = 1.0 / (theta ** (np.arange(half, dtype=np.float64) * 2.0 / head_dim))
    wavelen = 2.0 * np.pi / freq
    low_wl, high_wl = old_ctx / low_ff, old_ctx / high_ff
    smooth = (old_ctx / wavelen - low_ff) / (high_ff - low_ff)
    out = np.where(
        wavelen < high_wl,
        freq,
        np.where(wavelen > low_wl, freq / scale, (1.0 - smooth) * freq / scale + smooth * freq),
    )
    return out.astype(np.float64)


def _rope_tables(pos: np.ndarray) -> tuple[np.ndarray, np.ndarray]:
    """cosF [128, n]: cos duplicated on both partition halves.
    sinF2 [128, n]: +sin on rows 0-63, -sin on rows 64-127. The kernel
    computes out = x*cosF + halfswap(x*sinF2), which equals rotate-half
    RoPE."""
    freqs = _llama31_freqs_np(HD)
    ang = pos.astype(np.float64)[None, :] * freqs[:, None]  # [64, n]
    c = np.cos(ang).astype(np.float32)
    s = np.sin(ang).astype(np.float32)
    cosF = np.concatenate([c, c], axis=0)
    sinF2 = np.concatenate([s, -s], axis=0)
    return np.ascontiguousarray(cosF), np.ascontiguousarray(sinF2)


def _build_program(split_kv: bool):
    nc = bacc.Bacc(
        "TRN2",
        target_bir_lowering=False,
        debug=False,
        enable_asserts=False,
        num_devices=NCORES,
    )
    hT = nc.dram_tensor("hT", [HID, T], BF16, kind="ExternalInput")
    hTkv = (
        nc.dram_tensor("hTkv", [HID, T], BF16, kind="ExternalInput") if split_kv else hT
    )
    wqT = nc.dram_tensor("wqT", [HID, DQ], BF16, kind="ExternalInput")
    wkT = nc.dram_tensor("wkT", [HID, HD], BF16, kind="ExternalInput")
    wvT = nc.dram_tensor("wvT", [HID, HD], BF16, kind="ExternalInput")
    woT = nc.dram_tensor("woT", [DQ, HID], BF16, kind="ExternalInput")
    # K (page-rank order) positions are always 0..S-1 per sequence; a
    # [128, S] table sliced modulo S covers both passes in the identity
    # case and the KV pass in the permuted case.
    coskv = nc.dram_tensor("coskv", [128, S], FP32, kind="ExternalInput")
    sinkv = nc.dram_tensor("sinkv", [128, S], FP32, kind="ExternalInput")
    if split_kv:
        cosq = nc.dram_tensor("cosq", [128, T], FP32, kind="ExternalInput")
        sinq = nc.dram_tensor("sinq", [128, T], FP32, kind="ExternalInput")
    trid = nc.dram_tensor("trid", [128, 128], BF16, kind="ExternalInput")
    onesd = nc.dram_tensor("onesd", [128, 128], BF16, kind="ExternalInput")
    identd = nc.dram_tensor("identd", [128, 128], BF16, kind="ExternalInput")
    outp = nc.dram_tensor("outp", [T, HID], BF16, kind="ExternalOutput")

    with tile.TileContext(nc) as tc, ExitStack() as ctx:
        const_pool = ctx.enter_context(tc.tile_pool(name="const", bufs=1))
        QT = const_pool.tile([128, GQ, T], BF16)        # 32KB/part
        KT = const_pool.tile([128, T], BF16)            # 8KB
        V = const_pool.tile([128, T // 128, HD], BF16)  # 8KB (token-major tiles)
        aoT = const_pool.tile([128, GQ, T], BF16)       # 32KB
        tri_sb = const_pool.tile([128, 128], BF16)
        ones_sb = const_pool.tile([128, 128], BF16)
        ident_sb = const_pool.tile([128, 128], BF16)
        coskv_sb = const_pool.tile([128, S], FP32)
        sinkv_sb = const_pool.tile([128, S], FP32)
        # All DMAs stay on the single sync HWDGE queue: Tile assigns DMA
        # completion-sem lanes round-robin across queues while FIFO order
        # only holds per queue, so cross-queue DMAs can satisfy a
        # consumer's lane-wait out of order (observed as a flaky stale
        # weight read). Latency is managed by emission order instead:
        # constants are emitted inside the first chunk, below.
        if split_kv:
            cosq_sb = const_pool.tile([128, T], FP32)
            sinq_sb = const_pool.tile([128, T], FP32)
        else:
            cosq_sb, sinq_sb = coskv_sb, sinkv_sb

        def emit_const_dmas():
            nc.sync.dma_start(tri_sb[:], trid.ap()[:, :])
            nc.sync.dma_start(ones_sb[:], onesd.ap()[:, :])
            nc.sync.dma_start(ident_sb[:], identd.ap()[:, :])
            nc.sync.dma_start(coskv_sb[:], coskv.ap()[:, :])
            nc.sync.dma_start(sinkv_sb[:], sinkv.ap()[:, :])
            if split_kv:
                nc.sync.dma_start(cosq_sb[:], cosq.ap()[:, :])
                nc.sync.dma_start(sinq_sb[:], sinq.ap()[:, :])

        def rope_out(ps, cos_a, sin_a, out_full, tpool, n):
            """out = ps*cos + halfswap(ps*sinF2). The half-swap is two
            partition-shifted SBUF->SBUF DMAs (TensorTensor operands must
            share a start partition)."""
            t1 = tpool.tile([128, n], FP32, tag="t1", name="t1")
            u = tpool.tile([128, n], FP32, tag="u", name="u")
            u2 = tpool.tile([128, n], FP32, tag="u2", name="u2")
            nc.vector.tensor_mul(t1[:], ps[:], cos_a)
            nc.vector.tensor_mul(u[:], ps[:], sin_a)
            nc.sync.dma_start(u2[0:64, :], u[64:128, :])
            nc.sync.dma_start(u2[64:128, :], u[0:64, :])
            nc.vector.tensor_add(out_full, t1[:], u2[:])

        # Q pass runs FIRST (its 4-rope chunk evacuation is the expensive
        # one; the KV pass's cheaper evacuation then gates phase 2's PSUM
        # handoff). The h pool is shared by both passes so the second
        # pass's h DMAs start while the first pass is still computing.
        h_r = hT.ap().rearrange("(ko p) t -> p ko t", p=128)
        hkv_r = hTkv.ap().rearrange("(ko p) t -> p ko t", p=128)
        wq_r = wqT.ap().rearrange("(ko p) d -> p ko d", p=128)
        wk_r = wkT.ap().rearrange("(ko p) d -> p ko d", p=128)
        wv_r = wvT.ap().rearrange("(ko p) d -> p ko d", p=128)
        with tc.tile_pool(name="wqkv", bufs=1) as wpool, \
             tc.tile_pool(name="h1", bufs=5 if not split_kv else 4) as hpool:
            wq_sb = wpool.tile([128, KT32, DQ], BF16)
            wk_sb = wpool.tile([128, KT32, HD], BF16)
            wv_sb = wpool.tile([128, KT32, HD], BF16)
            # Startup streaming: wq k-groups are interleaved with chunk
            # 0's h tiles in sync-queue FIFO order, so the first matmuls
            # start after ~2MB instead of after all weights. Constants
            # land after chunk 0's h stream (needed at the first rope),
            # wk/wv after chunk 1's (needed a whole pass later).
            nc.sync.dma_start(wq_sb[:, 0:4, :], wq_r[:, 0:4, :])

            # ------------- phase 1b: Q projections -----------------------
            with tc.tile_pool(name="ps1b", bufs=2, space="PSUM") as ppool, \
                 tc.tile_pool(name="st1b", bufs=2 if not split_kv else 1) as stpool:
                for c in range(NQCH):
                    tsl = slice(c * QCH, (c + 1) * QCH)
                    # Chunk 0 streams at finer granularity, alternating wq
                    # k-groups with h pieces so the first matmuls start as
                    # early as possible.
                    kper = 4 if c == 0 else 8
                    hts = []
                    for j in range(32 // kper):
                        ht = hpool.tile([128, kper, QCH], BF16, tag="h", name="h")
                        nc.sync.dma_start(
                            ht[:], h_r[:, j * kper:(j + 1) * kper, tsl]
                        )
                        hts.append(ht)
                        if c == 0 and j + 1 < 8:
                            nc.sync.dma_start(
                                wq_sb[:, 4 * (j + 1):4 * (j + 1) + 4, :],
                                wq_r[:, 4 * (j + 1):4 * (j + 1) + 4, :],
                            )
                    if c == 0:
                        emit_const_dmas()
                    elif c == 1:
                        for kg in range(0, KT32, 8):
                            nc.sync.dma_start(
                                wk_sb[:, kg:kg + 8, :], wk_r[:, kg:kg + 8, :]
                            )
                            nc.sync.dma_start(
                                wv_sb[:, kg:kg + 8, :], wv_r[:, kg:kg + 8, :]
                            )
                    if split_kv:
                        cs, ss = cosq_sb[:, tsl], sinq_sb[:, tsl]
                    else:
                        p0 = (c * QCH) % S
                        cs, ss = cosq_sb[:, p0:p0 + QCH], sinq_sb[:, p0:p0 + QCH]
                    # The last chunk runs as two 2-head sub-passes so its
                    # final rope evacuation overlaps the second sub-pass's
                    # matmuls instead of stalling the next phase's PSUM
                    # handoff.
                    groups = (
                        [tuple(range(GQ))] if c < NQCH - 1 else [(0, 1), (2, 3)]
                    )
                    if c == 0:
                        # Preload the scalar engine's Exp spline tables off
                        # the critical path (first use otherwise stalls the
                        # attention phase ~2.7us).
                        warm = stpool.tile([128, 1], FP32, tag="warm", name="warm")
                        nc.scalar.activation(
                            warm[:], coskv_sb[:, 0:1],
                            mybir.ActivationFunctionType.Exp,
                        )
                    for gs in groups:
                        psq = {
                            g: ppool.tile(
                                [128, QCH], FP32, tag=f"psq{g}", name=f"psq{g}"
                            )
                            for g in gs
                        }
                        for k in range(KT32):
                            ht = hts[k // kper][:, k % kper, :]
                            st = k == 0
                            sp = k == KT32 - 1
                            for g in gs:
                                nc.tensor.matmul(
                                    psq[g][:], wq_sb[:, k, g * 128:(g + 1) * 128],
                                    ht, start=st, stop=sp,
                                )
                        for g in gs:
                            rope_out(psq[g], cs, ss, QT[:, g, tsl], stpool, QCH)

            # ------------- phase 1a: K/V projections ---------------------
            with tc.tile_pool(name="ps1a", bufs=2, space="PSUM") as ppool, \
                 tc.tile_pool(name="st1a", bufs=2 if not split_kv else 1) as stpool:
                for c in range(NQCH):
                    tsl = slice(c * QCH, (c + 1) * QCH)
                    hts = []
                    for j in range(4):
                        ht = hpool.tile([128, 8, QCH], BF16, tag="h", name="h")
                        nc.sync.dma_start(ht[:], hkv_r[:, j * 8:(j + 1) * 8, tsl])
                        hts.append(ht)
                    p0 = (c * QCH) % S

                    def emit_k(psk):
                        rope_out(
                            psk, coskv_sb[:, p0:p0 + QCH], sinkv_sb[:, p0:p0 + QCH],
                            KT[:, tsl], stpool, QCH,
                        )

                    def emit_v(psv):
                        vsb = stpool.tile([128, QCH], BF16, tag="vsb", name="vsb")
                        nc.scalar.copy(vsb[:], psv[:])
                        pst = ppool.tile([128, 4, 128], BF16, tag="pst", name="pst")
                        for i in range(4):
                            nc.tensor.transpose(
                                pst[:, i, :], vsb[:, i * 128:(i + 1) * 128],
                                ident_sb[:],
                            )
                            nc.vector.tensor_copy(V[:, 4 * c + i, :], pst[:, i, :])

                    if c < NQCH - 1:
                        psk = ppool.tile([128, QCH], FP32, tag="psk", name="psk")
                        psv = ppool.tile([128, QCH], FP32, tag="psv", name="psv")
                        for k in range(KT32):
                            ht = hts[k // 8][:, k % 8, :]
                            st = k == 0
                            sp = k == KT32 - 1
                            nc.tensor.matmul(
                                psk[:], wk_sb[:, k, :], ht, start=st, stop=sp
                            )
                            nc.tensor.matmul(
                                psv[:], wv_sb[:, k, :], ht, start=st, stop=sp
                            )
                        emit_k(psk)
                        emit_v(psv)
                    else:
                        # Last chunk: K sweep then V sweep, so the K rope
                        # overlaps the V matmuls and only the short V
                        # evacuation gates the attention phase.
                        psk = ppool.tile([128, QCH], FP32, tag="psk", name="psk")
                        for k in range(KT32):
                            nc.tensor.matmul(
                                psk[:], wk_sb[:, k, :], hts[k // 8][:, k % 8, :],
                                start=(k == 0), stop=(k == KT32 - 1),
                            )
                        emit_k(psk)
                        psv = ppool.tile([128, QCH], FP32, tag="psv", name="psv")
                        for k in range(KT32):
                            nc.tensor.matmul(
                                psv[:], wv_sb[:, k, :], hts[k // 8][:, k % 8, :],
                                start=(k == 0), stop=(k == KT32 - 1),
                            )
                        emit_v(psv)

        # ------------- phases 2+3, interleaved per batch ------------------
        wo_r = woT.ap().rearrange("(g p) e -> p g e", p=128)
        with tc.tile_pool(name="wo", bufs=1) as wopool:
            wo_sb = wopool.tile([128, GQ, HID], BF16)
            for g in range(GQ):
                nc.sync.dma_start(wo_sb[:, g, :], wo_r[:, g, :])

            with tc.tile_pool(name="sb2", bufs=2) as sbpool, \
                 tc.tile_pool(name="ex2", bufs=10) as expool, \
                 tc.tile_pool(name="pss2", bufs=2, space="PSUM") as pspool, \
                 tc.tile_pool(name="pv2", bufs=2, space="PSUM") as pvpool, \
                 tc.tile_pool(name="psd2", bufs=2, space="PSUM") as pdpool, \
                 tc.tile_pool(name="ps3", bufs=2, space="PSUM") as p3pool, \
                 tc.tile_pool(name="ob3", bufs=3) as obpool:
                def p2_unit(b, g, it):
                    qoff = it * QCH
                    q0 = b * S + qoff
                    njt = (qoff + QCH) // 128
                    es = sbpool.tile([128, QCH], BF16, tag="es", name="es")
                    pv = pvpool.tile([128, QCH], FP32, tag="pv", name="pv")
                    exs, offs = [], []
                    # All score matmuls stream first (the scalar engine's
                    # exps run one behind), then all PV matmuls — the PE
                    # never waits on the exp chain.
                    for jt in range(njt):
                        ko = b * S + jt * 128
                        off = jt * 128 - qoff if jt * 128 >= qoff else 0
                        pss = pspool.tile([128, QCH], FP32, tag="pss", name="pss")
                        nc.tensor.matmul(
                            pss[:, off:QCH],
                            KT[:, ko:ko + 128],
                            QT[:, g, q0 + off:q0 + QCH],
                            start=True, stop=True,
                        )
                        ex = expool.tile([128, QCH], BF16, tag="ex", name="ex")
                        nc.scalar.activation(
                            ex[:, off:QCH], pss[:, off:QCH],
                            mybir.ActivationFunctionType.Exp, scale=SCALE,
                        )
                        if jt * 128 >= qoff:  # diagonal block
                            nc.vector.tensor_mul(
                                ex[:, off:off + 128], ex[:, off:off + 128],
                                tri_sb[:],
                            )
                        if jt == 0:
                            nc.vector.tensor_copy(es[:], ex[:])
                        else:
                            nc.vector.tensor_add(
                                es[:, off:QCH], es[:, off:QCH], ex[:, off:QCH]
                            )
                        exs.append(ex)
                        offs.append(off)
                    for jt in range(njt):
                        nc.tensor.matmul(
                            pv[:, offs[jt]:QCH],
                            V[:, b * 8 + jt, :],
                            exs[jt][:, offs[jt]:QCH],
                            start=(jt == 0), stop=(jt == njt - 1),
                        )
                    psd = pdpool.tile([128, QCH], FP32, tag="psd", name="psd")
                    nc.tensor.matmul(psd[:], ones_sb[:], es[:], start=True, stop=True)
                    rec = sbpool.tile([128, QCH], FP32, tag="rec", name="rec")
                    nc.vector.reciprocal_approx_fast(rec[:], psd[:])
                    nc.vector.tensor_mul(aoT[:, g, q0:q0 + QCH], pv[:], rec[:])

                def p3_unit(eg, tb):
                    ob = obpool.tile([128, 4, QCH], BF16, tag="ob", name="ob")
                    for ei in range(4):
                        e0 = eg * 2048 + ei * QCH
                        pso = p3pool.tile([128, QCH], FP32, tag="pso", name="pso")
                        for g in range(GQ):
                            nc.tensor.matmul(
                                pso[:],
                                aoT[:, g, tb * 128:(tb + 1) * 128],
                                wo_sb[:, g, e0:e0 + QCH],
                                start=(g == 0), stop=(g == GQ - 1),
                            )
                        if ei % 2 == 0:
                            nc.scalar.copy(ob[:, ei, :], pso[:])
                        else:
                            nc.vector.tensor_copy(ob[:, ei, :], pso[:])
                        if ei == 1:
                            nc.sync.dma_start(
                                outp.ap()[tb * 128:(tb + 1) * 128,
                                          eg * 2048:eg * 2048 + 1024],
                                ob[:, 0:2, :],
                            )
                    nc.sync.dma_start(
                        outp.ap()[tb * 128:(tb + 1) * 128,
                                  eg * 2048 + 1024:(eg + 1) * 2048],
                        ob[:, 2:4, :],
                    )

                # Batch b's attention units are interleaved with batch
                # b-1's output-projection units: the projection's dense
                # matmul stream fills the PE whenever attention waits on
                # the exp chain.
                for b in range(B):
                    fill = (
                        [(eg, tb) for eg in range(2)
                         for tb in range(8 * (b - 1), 8 * (b - 1) + 8)]
                        if b > 0 else []
                    )
                    ui = 0
                    for g in range(GQ):
                        for it in range(2):
                            p2_unit(b, g, it)
                            for eg, tb in fill[2 * ui:2 * ui + 2]:
                                p3_unit(eg, tb)
                            ui += 1
                for eg in range(2):
                    for tb in range(8 * (B - 1), 8 * (B - 1) + 8):
                        p3_unit(eg, tb)

    nc.finalize()
    return nc


def _get_program(split_kv: bool):
    if split_kv not in _PROG_CACHE:
        _PROG_CACHE[split_kv] = _build_program(split_kv)
    return _PROG_CACHE[split_kv]


def kernel(
    hidden_states, wq, wk, wv, wo, kv_cache, position_ids,
    kv_page_indices, kv_page_indptr, kv_last_page_lens, qo_indptr,
    _run_kwargs: dict | None = None,
):
    hidden_states = np.asarray(hidden_states, np.float32)
    wq = np.asarray(wq, np.float32)
    wk = np.asarray(wk, np.float32)
    wv = np.asarray(wv, np.float32)
    wo = np.asarray(wo, np.float32)
    position_ids = np.asarray(position_ids, np.int32)
    qo_indptr = np.asarray(qo_indptr, np.int64)

    nnz = hidden_states.shape[0]
    b = qo_indptr.shape[0] - 1
    assert nnz == T and b == B, (nnz, b)
    assert np.array_equal(qo_indptr, np.arange(B + 1, dtype=np.int64) * S), (
        "kernel assumes uniform sequence lengths of 1024"
    )

    # Page-gather order: the reference gathers pages in list order, so the
    # token with position p within its sequence lands at page-order rank p.
    # KV must be fed in rank order; the q path stays in token order.
    perm = np.empty(T, np.int64)
    identity = True
    for bi in range(B):
        pos_b = position_ids[bi * S:(bi + 1) * S].astype(np.int64)
        assert np.array_equal(np.sort(pos_b), np.arange(S)), (
            "kernel assumes positions cover 0..S-1 exactly once per sequence"
        )
        inv = np.empty(S, np.int64)
        inv[pos_b] = np.arange(S)
        perm[bi * S:(bi + 1) * S] = bi * S + inv
        if not np.array_equal(inv, np.arange(S)):
            identity = False

    hT16 = np.ascontiguousarray(hidden_states.T.astype(NP_BF16))
    coskv, sinkv = _rope_tables(np.arange(S, dtype=np.int64))
    tri = np.ascontiguousarray(
        (np.arange(128)[:, None] <= np.arange(128)[None, :]).astype(NP_BF16)
    )
    ones = np.ones((128, 128), NP_BF16)
    eye = np.eye(128, dtype=np.float32).astype(NP_BF16)

    split_kv = not identity
    nc = _get_program(split_kv)

    in_maps = []
    for c in range(NCORES):
        im = {
            "hT": hT16,
            "wqT": np.ascontiguousarray(wq[c * DQ:(c + 1) * DQ, :].T.astype(NP_BF16)),
            "wkT": np.ascontiguousarray(wk[c * HD:(c + 1) * HD, :].T.astype(NP_BF16)),
            "wvT": np.ascontiguousarray(wv[c * HD:(c + 1) * HD, :].T.astype(NP_BF16)),
            "woT": np.ascontiguousarray(wo[:, c * DQ:(c + 1) * DQ].T.astype(NP_BF16)),
            "coskv": coskv,
            "sinkv": sinkv,
            "trid": tri,
            "onesd": ones,
            "identd": eye,
        }
        if split_kv:
            im["hTkv"] = np.ascontiguousarray(hT16[:, perm])
            cosq, sinq = _rope_tables(position_ids)
            im["cosq"] = cosq
            im["sinq"] = sinq
        in_maps.append(im)

    res = run_bass_kernel_spmd(
        nc, in_maps, core_ids=list(range(NCORES)), **(_run_kwargs or {})
    )
    out = np.zeros((T, HID), np.float32)
    for c in range(NCORES):
        out += res.results[c]["outp"].astype(np.float32)
    kernel.last_results = res  # type: ignore[attr-defined]
    return out


# revision 28
# speedup vs baseline: 1.1285x; 1.0004x over previous
"""Trainium2 Bass kernel for paged-attention Llama-style block (nn_L4maAttention).

Sharding: tensor-parallel over heads across 8 NeuronCores. Core c owns
q-heads [4c, 4c+4), kv-head c, wq/wk/wv row shards and the matching wo
column shard. Each core computes a full [T, HID] partial of the output
projection in bf16; the host sums the 8 partials (the TP reduce).

Device kernel (per core), matmuls in bf16 (full PE rate, half the DMA
of fp32), fp32 PSUM accumulation. All DMAs stay on the single sync
HWDGE queue (see the in-code note on cross-queue completion-sem
ordering); startup latency is managed by emission order instead.

  phase 1b (Q pass, first): Q projections in 8 token chunks of 512,
        4 heads x [128,512] PSUM (4 banks x 2 buffers = all 8), RoPE
        -> QT (SBUF resident, bf16). RoPE computes x*cos + swap*sinF2
        with the half-swap done by two partition-shifted SBUF->SBUF
        DMAs. Chunk 0 interleaves wq k-group DMAs with its h pieces so
        the first matmul starts after ~1MB; the exp activation table
        is preloaded here. The last chunk runs as two 2-head
        sub-passes so its rope evacuation overlaps matmuls.
  phase 1a (KV pass): K/V projections, same chunking, PSUM psk + psv
        + transpose bank, x2 buffers = 6 banks. K^T + RoPE -> KT;
        V computed head-dim-major then PE-transposed (identity
        matmul) to token-major tiles in V. Last chunk splits into a
        K sweep then V sweep so only the short V evacuation gates the
        attention phase's PSUM handoff.
  phases 2+3, interleaved per batch: attention unit (b,g,it) =
        transposed scores [k on partitions, q free] over the causal
        suffix of each 128-row k block; exp in bf16 on the scalar
        engine with 1/sqrt(d) folded into the activation scale;
        single [128,128] triangular mask on the diagonal sub-block;
        all score matmuls stream first, then all PV matmuls, so the
        PE never waits on the exp chain; denominators via a
        ones-matmul (broadcast over partitions) + fast reciprocal.
        Batch b-1's output-projection units (aoT.T @ woT, evacuation
        alternating scalar/vector, split 256KB output DMAs) are
        emitted between batch b's attention units as dense PE filler.
"""

import math
import sys
from contextlib import ExitStack

import numpy as np

for _p in ("/opt/trn_rl_repo",):
    if _p not in sys.path:
        sys.path.insert(0, _p)

import concourse.mybir as mybir  # noqa: E402
import concourse.tile as tile  # noqa: E402
from concourse import bacc  # noqa: E402
from concourse.bass_utils import run_bass_kernel_spmd  # noqa: E402

NCORES = 8
HID = 4096
NH = 32
NKV = 8
HD = 128
B = 4
S = 1024
T = B * S
GQ = NH // NCORES          # q heads per core = 4
DQ = GQ * HD               # 512
KT32 = HID // 128          # 32 k tiles
QCH = 512                  # token chunk
NQCH = T // QCH            # 8
SCALE = 1.0 / math.sqrt(HD)

FP32 = mybir.dt.float32
BF16 = mybir.dt.bfloat16
NP_BF16 = mybir.dt.np(BF16)

_PROG_CACHE: dict = {}


def _llama31_freqs_np(head_dim: int) -> np.ndarray:
    half = head_dim // 2
    theta, scale, low_ff, high_ff, old_ctx = 500000.0, 8.0, 1.0, 4.0, 8192.0
    freq = 1.0 / (theta ** (np.arange(half, dtype=np.float64) * 2.0 / head_dim))
    wavelen = 2.0 * np.pi / freq
    low_wl, high_wl = old_ctx / low_ff, old_ctx / high_ff
    smooth = (old_ctx / wavelen - low_ff) / (high_ff - low_ff)
    out = np.where(
        wavelen < high_wl,
        freq,
        np.where(wavelen > low_wl, freq / scale, (1.0 - smooth) * freq / scale + smooth * freq),
    )
    return out.astype(np.float64)


def _rope_tables(pos: np.ndarray) -> tuple[np.ndarray, np.ndarray]:
    """cosF [128, n]: cos duplicated on both partition halves.
    sinF2 [128, n]: +sin on rows 0-63, -sin on rows 64-127. The kernel
    computes out = x*cosF + halfswap(x*sinF2), which equals rotate-half
    RoPE."""
    freqs = _llama31_freqs_np(HD)
    ang = pos.astype(np.float64)[None, :] * freqs[:, None]  # [64, n]
    c = np.cos(ang).astype(np.float32)
    s = np.sin(ang).astype(np.float32)
    cosF = np.concatenate([c, c], axis=0)
    sinF2 = np.concatenate([s, -s], axis=0)
    return np.ascontiguousarray(cosF), np.ascontiguousarray(sinF2)


def _build_program(split_kv: bool):
    nc = bacc.Bacc(
        "TRN2",
        target_bir_lowering=False,
        debug=False,
        enable_asserts=False,
        num_devices=NCORES,
    )
    hT = nc.dram_tensor("hT", [HID, T], BF16, kind="ExternalInput")
    hTkv = (
        nc.dram_tensor("hTkv", [HID, T], BF16, kind="ExternalInput") if split_kv else hT
    )
    wqT = nc.dram_tensor("wqT", [HID, DQ], BF16, kind="ExternalInput")
    wkT = nc.dram_tensor("wkT", [HID, HD], BF16, kind="ExternalInput")
    wvT = nc.dram_tensor("wvT", [HID, HD], BF16, kind="ExternalInput")
    woT = nc.dram_tensor("woT", [DQ, HID], BF16, kind="ExternalInput")
    # K (page-rank order) positions are always 0..S-1 per sequence; a
    # [128, S] table sliced modulo S covers both passes in the identity
    # case and the KV pass in the permuted case.
    coskv = nc.dram_tensor("coskv", [128, S], FP32, kind="ExternalInput")
    sinkv = nc.dram_tensor("sinkv", [128, S], FP32, kind="ExternalInput")
    if split_kv:
        cosq = nc.dram_tensor("cosq", [128, T], FP32, kind="ExternalInput")
        sinq = nc.dram_tensor("sinq", [128, T], FP32, kind="ExternalInput")
    trid = nc.dram_tensor("trid", [128, 128], BF16, kind="ExternalInput")
    onesd = nc.dram_tensor("onesd", [128, 128], BF16, kind="ExternalInput")
    identd = nc.dram_tensor("identd", [128, 128], BF16, kind="ExternalInput")
    outp = nc.dram_tensor("outp", [T, HID], BF16, kind="ExternalOutput")

    with tile.TileContext(nc) as tc, ExitStack() as ctx:
        const_pool = ctx.enter_context(tc.tile_pool(name="const", bufs=1))
        QT = const_pool.tile([128, GQ, T], BF16)        # 32KB/part
        KT = const_pool.tile([128, T], BF16)            # 8KB
        V = const_pool.tile([128, T // 128, HD], BF16)  # 8KB (token-major tiles)
        aoT = const_pool.tile([128, GQ, T], BF16)       # 32KB
        tri_sb = const_pool.tile([128, 128], BF16)
        ones_sb = const_pool.tile([128, 128], BF16)
        ident_sb = const_pool.tile([128, 128], BF16)
        coskv_sb = const_pool.tile([128, S], FP32)
        sinkv_sb = const_pool.tile([128, S], FP32)
        # All DMAs stay on the single sync HWDGE queue: Tile assigns DMA
        # completion-sem lanes round-robin across queues while FIFO order
        # only holds per queue, so cross-queue DMAs can satisfy a
        # consumer's lane-wait out of order (observed as a flaky stale
        # weight read). Latency is managed by emission order instead:
        # constants are emitted inside the first chunk, below.
        if split_kv:
            cosq_sb = const_pool.tile([128, T], FP32)
            sinq_sb = const_pool.tile([128, T], FP32)
        else:
            cosq_sb, sinq_sb = coskv_sb, sinkv_sb

        def emit_const_dmas():
            nc.sync.dma_start(tri_sb[:], trid.ap()[:, :])
            nc.sync.dma_start(ones_sb[:], onesd.ap()[:, :])
            nc.sync.dma_start(ident_sb[:], identd.ap()[:, :])
            nc.sync.dma_start(coskv_sb[:], coskv.ap()[:, :])
            nc.sync.dma_start(sinkv_sb[:], sinkv.ap()[:, :])
            if split_kv:
                nc.sync.dma_start(cosq_sb[:], cosq.ap()[:, :])
                nc.sync.dma_start(sinq_sb[:], sinq.ap()[:, :])

        def rope_out(ps, cos_a, sin_a, out_full, tpool, n):
            """out = ps*cos + halfswap(ps*sinF2). The half-swap is two
            partition-shifted SBUF->SBUF DMAs (TensorTensor operands must
            share a start partition)."""
            t1 = tpool.tile([128, n], FP32, tag="t1", name="t1")
            u = tpool.tile([128, n], FP32, tag="u", name="u")
            u2 = tpool.tile([128, n], FP32, tag="u2", name="u2")
            nc.vector.tensor_mul(t1[:], ps[:], cos_a)
            nc.vector.tensor_mul(u[:], ps[:], sin_a)
            nc.sync.dma_start(u2[0:64, :], u[64:128, :])
            nc.sync.dma_start(u2[64:128, :], u[0:64, :])
            nc.vector.tensor_add(out_full, t1[:], u2[:])

        # Q pass runs FIRST (its 4-rope chunk evacuation is the expensive
        # one; the KV pass's cheaper evacuation then gates phase 2's PSUM
        # handoff). The h pool is shared by both passes so the second
        # pass's h DMAs start while the first pass is still computing.
        h_r = hT.ap().rearrange("(ko p) t -> p ko t", p=128)
        hkv_r = hTkv.ap().rearrange("(ko p) t -> p ko t", p=128)
        wq_r = wqT.ap().rearrange("(ko p) d -> p ko d", p=128)
        wk_r = wkT.ap().rearrange("(ko p) d -> p ko d", p=128)
        wv_r = wvT.ap().rearrange("(ko p) d -> p ko d", p=128)
        with tc.tile_pool(name="wqkv", bufs=1) as wpool, \
             tc.tile_pool(name="h1", bufs=5 if not split_kv else 4) as hpool:
            wq_sb = wpool.tile([128, KT32, DQ], BF16)
            wk_sb = wpool.tile([128, KT32, HD], BF16)
            wv_sb = wpool.tile([128, KT32, HD], BF16)
            # Startup streaming: wq k-groups are interleaved with chunk
            # 0's h tiles in sync-queue FIFO order, so the first matmuls
            # start after ~2MB instead of after all weights. Constants
            # land after chunk 0's h stream (needed at the first rope),
            # wk/wv after chunk 1's (needed a whole pass later).
            nc.sync.dma_start(wq_sb[:, 0:4, :], wq_r[:, 0:4, :])

            # ------------- phase 1b: Q projections -----------------------
            with tc.tile_pool(name="ps1b", bufs=2, space="PSUM") as ppool, \
                 tc.tile_pool(name="st1b", bufs=2 if not split_kv else 1) as stpool:
                for c in range(NQCH):
                    tsl = slice(c * QCH, (c + 1) * QCH)
                    # Chunk 0 streams at finer granularity, alternating wq
                    # k-groups with h pieces so the first matmuls start as
                    # early as possible.
                    kper = 4 if c == 0 else 8
                    hts = []
                    for j in range(32 // kper):
                        ht = hpool.tile([128, kper, QCH], BF16, tag="h", name="h")
                        nc.sync.dma_start(
                            ht[:], h_r[:, j * kper:(j + 1) * kper, tsl]
                        )
                        hts.append(ht)
                        if c == 0 and j + 1 < 8:
                            nc.sync.dma_start(
                                wq_sb[:, 4 * (j + 1):4 * (j + 1) + 4, :],
                                wq_r[:, 4 * (j + 1):4 * (j + 1) + 4, :],
                            )
                    if c == 0:
                        emit_const_dmas()
                    elif c == 1:
                        for kg in range(0, KT32, 8):
                            nc.sync.dma_start(
                                wk_sb[:, kg:kg + 8, :], wk_r[:, kg:kg + 8, :]
                            )
                            nc.sync.dma_start(
                                wv_sb[:, kg:kg + 8, :], wv_r[:, kg:kg + 8, :]
                            )
                    if split_kv:
                        cs, ss = cosq_sb[:, tsl], sinq_sb[:, tsl]
                    else:
                        p0 = (c * QCH) % S
                        cs, ss = cosq_sb[:, p0:p0 + QCH], sinq_sb[:, p0:p0 + QCH]
                    # The last chunk runs as two 2-head sub-passes so its
                    # final rope evacuation overlaps the second sub-pass's
                    # matmuls instead of stalling the next phase's PSUM
                    # handoff.
                    groups = (
                        [tuple(range(GQ))] if c < NQCH - 1 else [(0, 1), (2, 3)]
                    )
                    if c == 0:
                        # Preload the scalar engine's Exp spline tables off
                        # the critical path (first use otherwise stalls the
                        # attention phase ~2.7us).
                        warm = stpool.tile([128, 1], FP32, tag="warm", name="warm")
                        nc.scalar.activation(
                            warm[:], coskv_sb[:, 0:1],
                            mybir.ActivationFunctionType.Exp,
                        )
                    for gs in groups:
                        psq = {
                            g: ppool.tile(
                                [128, QCH], FP32, tag=f"psq{g}", name=f"psq{g}"
                            )
                            for g in gs
                        }
                        for k in range(KT32):
                            ht = hts[k // kper][:, k % kper, :]
                            st = k == 0
                            sp = k == KT32 - 1
                            for g in gs:
                                nc.tensor.matmul(
                                    psq[g][:], wq_sb[:, k, g * 128:(g + 1) * 128],
                                    ht, start=st, stop=sp,
                                )
                        for g in gs:
                            rope_out(psq[g], cs, ss, QT[:, g, tsl], stpool, QCH)

            # ------------- phase 1a: K/V projections ---------------------
            with tc.tile_pool(name="ps1a", bufs=2, space="PSUM") as ppool, \
                 tc.tile_pool(name="st1a", bufs=2 if not split_kv else 1) as stpool:
                for c in range(NQCH):
                    tsl = slice(c * QCH, (c + 1) * QCH)
                    hts = []
                    for j in range(4):
                        ht = hpool.tile([128, 8, QCH], BF16, tag="h", name="h")
                        nc.sync.dma_start(ht[:], hkv_r[:, j * 8:(j + 1) * 8, tsl])
                        hts.append(ht)
                    p0 = (c * QCH) % S

                    def emit_k(psk):
                        rope_out(
                            psk, coskv_sb[:, p0:p0 + QCH], sinkv_sb[:, p0:p0 + QCH],
                            KT[:, tsl], stpool, QCH,
                        )

                    def emit_v(psv):
                        vsb = stpool.tile([128, QCH], BF16, tag="vsb", name="vsb")
                        nc.scalar.copy(vsb[:], psv[:])
                        pst = ppool.tile([128, 4, 128], BF16, tag="pst", name="pst")
                        for i in range(4):
                            nc.tensor.transpose(
                                pst[:, i, :], vsb[:, i * 128:(i + 1) * 128],
                                ident_sb[:],
                            )
                            nc.vector.tensor_copy(V[:, 4 * c + i, :], pst[:, i, :])

                    if c < NQCH - 1:
                        psk = ppool.tile([128, QCH], FP32, tag="psk", name="psk")
                        psv = ppool.tile([128, QCH], FP32, tag="psv", name="psv")
                        for k in range(KT32):
                            ht = hts[k // 8][:, k % 8, :]
                            st = k == 0
                            sp = k == KT32 - 1
                            nc.tensor.matmul(
                                psk[:], wk_sb[:, k, :], ht, start=st, stop=sp
                            )
                            nc.tensor.matmul(
                                psv[:], wv_sb[:, k, :], ht, start=st, stop=sp
                            )
                        emit_k(psk)
                        emit_v(psv)
                    else:
                        # Last chunk: K sweep then V sweep, so the K rope
                        # overlaps the V matmuls and only the short V
                        # evacuation gates the attention phase.
                        psk = ppool.tile([128, QCH], FP32, tag="psk", name="psk")
                        for k in range(KT32):
                            nc.tensor.matmul(
                                psk[:], wk_sb[:, k, :], hts[k // 8][:, k % 8, :],
                                start=(k == 0), stop=(k == KT32 - 1),
                            )
                        emit_k(psk)
                        psv = ppool.tile([128, QCH], FP32, tag="psv", name="psv")
                        for k in range(KT32):
                            nc.tensor.matmul(
                                psv[:], wv_sb[:, k, :], hts[k // 8][:, k % 8, :],
                                start=(k == 0), stop=(k == KT32 - 1),
                            )
                        emit_v(psv)

        # ------------- phases 2+3, interleaved per batch ------------------
        wo_r = woT.ap().rearrange("(g p) e -> p g e", p=128)
        with tc.tile_pool(name="wo", bufs=1) as wopool:
            wo_sb = wopool.tile([128, GQ, HID], BF16)
            for g in range(GQ):
                nc.sync.dma_start(wo_sb[:, g, :], wo_r[:, g, :])

            with tc.tile_pool(name="sb2", bufs=2) as sbpool, \
                 tc.tile_pool(name="ex2", bufs=10) as expool, \
                 tc.tile_pool(name="pss2", bufs=2, space="PSUM") as pspool, \
                 tc.tile_pool(name="pv2", bufs=2, space="PSUM") as pvpool, \
                 tc.tile_pool(name="psd2", bufs=2, space="PSUM") as pdpool, \
                 tc.tile_pool(name="ps3", bufs=2, space="PSUM") as p3pool, \
                 tc.tile_pool(name="ob3", bufs=3) as obpool:
                def p2_unit(b, g, it):
                    qoff = it * QCH
                    q0 = b * S + qoff
                    njt = (qoff + QCH) // 128
                    es = sbpool.tile([128, QCH], BF16, tag="es", name="es")
                    pv = pvpool.tile([128, QCH], FP32, tag="pv", name="pv")
                    exs, offs = [], []
                    # All score matmuls stream first (the scalar engine's
                    # exps run one behind), then all PV matmuls — the PE
                    # never waits on the exp chain.
                    for jt in range(njt):
                        ko = b * S + jt * 128
                        off = jt * 128 - qoff if jt * 128 >= qoff else 0
                        pss = pspool.tile([128, QCH], FP32, tag="pss", name="pss")
                        nc.tensor.matmul(
                            pss[:, off:QCH],
                            KT[:, ko:ko + 128],
                            QT[:, g, q0 + off:q0 + QCH],
                            start=True, stop=True,
                        )
                        ex = expool.tile([128, QCH], BF16, tag="ex", name="ex")
                        nc.scalar.activation(
                            ex[:, off:QCH], pss[:, off:QCH],
                            mybir.ActivationFunctionType.Exp, scale=SCALE,
                        )
                        if jt * 128 >= qoff:  # diagonal block
                            nc.vector.tensor_mul(
                                ex[:, off:off + 128], ex[:, off:off + 128],
                                tri_sb[:],
                            )
                        if jt == 0:
                            nc.vector.tensor_copy(es[:], ex[:])
                        else:
                            nc.vector.tensor_add(
                                es[:, off:QCH], es[:, off:QCH], ex[:, off:QCH]
                            )
                        exs.append(ex)
                        offs.append(off)
                    for jt in range(njt):
                        nc.tensor.matmul(
                            pv[:, offs[jt]:QCH],
                            V[:, b * 8 + jt, :],
                            exs[jt][:, offs[jt]:QCH],
                            start=(jt == 0), stop=(jt == njt - 1),
                        )
                    psd = pdpool.tile([128, QCH], FP32, tag="psd", name="psd")
                    nc.tensor.matmul(psd[:], ones_sb[:], es[:], start=True, stop=True)
                    rec = sbpool.tile([128, QCH], FP32, tag="rec", name="rec")
                    nc.vector.reciprocal_approx_fast(rec[:], psd[:])
                    nc.vector.tensor_mul(aoT[:, g, q0:q0 + QCH], pv[:], rec[:])

                def p3_unit(eg, tb):
                    ob = obpool.tile([128, 4, QCH], BF16, tag="ob", name="ob")
                    for ei in range(4):
                        e0 = eg * 2048 + ei * QCH
                        pso = p3pool.tile([128, QCH], FP32, tag="pso", name="pso")
                        for g in range(GQ):
                            nc.tensor.matmul(
                                pso[:],
                                aoT[:, g, tb * 128:(tb + 1) * 128],
                                wo_sb[:, g, e0:e0 + QCH],
                                start=(g == 0), stop=(g == GQ - 1),
                            )
                        if ei % 2 == 0:
                            nc.scalar.copy(ob[:, ei, :], pso[:])
                        else:
                            nc.vector.tensor_copy(ob[:, ei, :], pso[:])
                        if ei == 1:
                            nc.sync.dma_start(
                                outp.ap()[tb * 128:(tb + 1) * 128,
                                          eg * 2048:eg * 2048 + 1024],
                                ob[:, 0:2, :],
                            )
                    nc.sync.dma_start(
                        outp.ap()[tb * 128:(tb + 1) * 128,
                                  eg * 2048 + 1024:(eg + 1) * 2048],
                        ob[:, 2:4, :],
                    )

                # Batch b's attention units are interleaved with batch
                # b-1's output-projection units: the projection's dense
                # matmul stream fills the PE whenever attention waits on
                # the exp chain.
                for b in range(B):
                    fill = (
                        [(eg, tb) for eg in range(2)
                         for tb in range(8 * (b - 1), 8 * (b - 1) + 8)]
                        if b > 0 else []
                    )
                    ui = 0
                    for g in range(GQ):
                        for it in range(2):
                            p2_unit(b, g, it)
                            for eg, tb in fill[2 * ui:2 * ui + 2]:
                                p3_unit(eg, tb)
                            ui += 1
                for eg in range(2):
                    for tb in range(8 * (B - 1), 8 * (B - 1) + 8):
                        p3_unit(eg, tb)

    nc.finalize()
    return nc


def _get_program(split_kv: bool):
    if split_kv not in _PROG_CACHE:
        _PROG_CACHE[split_kv] = _build_program(split_kv)
    return _PROG_CACHE[split_kv]


def kernel(
    hidden_states, wq, wk, wv, wo, kv_cache, position_ids,
    kv_page_indices, kv_page_indptr, kv_last_page_lens, qo_indptr,
    _run_kwargs: dict | None = None,
):
    hidden_states = np.asarray(hidden_states, np.float32)
    wq = np.asarray(wq, np.float32)
    wk = np.asarray(wk, np.float32)
    wv = np.asarray(wv, np.float32)
    wo = np.asarray(wo, np.float32)
    position_ids = np.asarray(position_ids, np.int32)
    qo_indptr = np.asarray(qo_indptr, np.int64)

    nnz = hidden_states.shape[0]
    b = qo_indptr.shape[0] - 1
    assert nnz == T and b == B, (nnz, b)
    assert np.array_equal(qo_indptr, np.arange(B + 1, dtype=np.int64) * S), (
        "kernel assumes uniform sequence lengths of 1024"
    )

    # Page-gather order: the reference gathers pages in list order, so the
    # token with position p within its sequence lands at page-order rank p.
    # KV must be fed in rank order; the q path stays in token order.
    perm = np.empty(T, np.int64)
    identity = True
    for bi in range(B):
        pos_b = position_ids[bi * S:(bi + 1) * S].astype(np.int64)
        assert np.array_equal(np.sort(pos_b), np.arange(S)), (
            "kernel assumes positions cover 0..S-1 exactly once per sequence"
        )
        inv = np.empty(S, np.int64)
        inv[pos_b] = np.arange(S)
        perm[bi * S:(bi + 1) * S] = bi * S + inv
        if not np.array_equal(inv, np.arange(S)):
            identity = False

    hT16 = np.ascontiguousarray(hidden_states.T.astype(NP_BF16))
    coskv, sinkv = _rope_tables(np.arange(S, dtype=np.int64))
    tri = np.ascontiguousarray(
        (np.arange(128)[:, None] <= np.arange(128)[None, :]).astype(NP_BF16)
    )
    ones = np.ones((128, 128), NP_BF16)
    eye = np.eye(128, dtype=np.float32).astype(NP_BF16)

    split_kv = not identity
    nc = _get_program(split_kv)

    in_maps = []
    for c in range(NCORES):
        im = {
            "hT": hT16,
            "wqT": np.ascontiguousarray(wq[c * DQ:(c + 1) * DQ, :].T.astype(NP_BF16)),
            "wkT": np.ascontiguousarray(wk[c * HD:(c + 1) * HD, :].T.astype(NP_BF16)),
            "wvT": np.ascontiguousarray(wv[c * HD:(c + 1) * HD, :].T.astype(NP_BF16)),
            "woT": np.ascontiguousarray(wo[:, c * DQ:(c + 1) * DQ].T.astype(NP_BF16)),
            "coskv": coskv,
            "sinkv": sinkv,
            "trid": tri,
            "onesd": ones,
            "identd": eye,
        }
        if split_kv:
            im["hTkv"] = np.ascontiguousarray(hT16[:, perm])
            cosq, sinq = _rope_tables(position_ids)
            im["cosq"] = cosq
            im["sinq"] = sinq
        in_maps.append(im)

    res = run_bass_kernel_spmd(
        nc, in_maps, core_ids=list(range(NCORES)), **(_run_kwargs or {})
    )
    out = np.zeros((T, HID), np.float32)
    for c in range(NCORES):
        out += res.results[c]["outp"].astype(np.float32)
    kernel.last_results = res  # type: ignore[attr-defined]
    return out


# revision 29
# speedup vs baseline: 1.1309x; 1.0022x over previous
"""Trainium2 Bass kernel for paged-attention Llama-style block (nn_L4maAttention).

Sharding: tensor-parallel over heads across 8 NeuronCores. Core c owns
q-heads [4c, 4c+4), kv-head c, wq/wk/wv row shards and the matching wo
column shard. Each core computes a full [T, HID] partial of the output
projection in bf16; the host sums the 8 partials (the TP reduce).

Device kernel (per core), matmuls in bf16 (full PE rate, half the DMA
of fp32), fp32 PSUM accumulation. All DMAs stay on the single sync
HWDGE queue (see the in-code note on cross-queue completion-sem
ordering); startup latency is managed by emission order instead.

  phase 1b (Q pass, first): Q projections in 8 token chunks of 512,
        4 heads x [128,512] PSUM (4 banks x 2 buffers = all 8), RoPE
        -> QT (SBUF resident, bf16). RoPE computes x*cos + swap*sinF2
        with the half-swap done by two partition-shifted SBUF->SBUF
        DMAs. Chunk 0 interleaves wq k-group DMAs with its h pieces so
        the first matmul starts after ~1MB; the exp activation table
        is preloaded here. The last chunk runs as two 2-head
        sub-passes so its rope evacuation overlaps matmuls.
  phase 1a (KV pass): K/V projections, same chunking, PSUM psk + psv
        + transpose bank, x2 buffers = 6 banks. K^T + RoPE -> KT;
        V computed head-dim-major then PE-transposed (identity
        matmul) to token-major tiles in V. Last chunk splits into a
        K sweep then V sweep so only the short V evacuation gates the
        attention phase's PSUM handoff.
  phases 2+3, interleaved per batch: attention unit (b,g,it) =
        transposed scores [k on partitions, q free] over the causal
        suffix of each 128-row k block; exp in bf16 on the scalar
        engine with 1/sqrt(d) folded into the activation scale;
        single [128,128] triangular mask on the diagonal sub-block;
        all score matmuls stream first, then all PV matmuls, so the
        PE never waits on the exp chain; denominators via a
        ones-matmul (broadcast over partitions) + fast reciprocal.
        Batch b-1's output-projection units (aoT.T @ woT, evacuation
        alternating scalar/vector, split 256KB output DMAs) are
        emitted between batch b's attention units as dense PE filler.
"""

import math
import sys
from contextlib import ExitStack

import numpy as np

for _p in ("/opt/trn_rl_repo",):
    if _p not in sys.path:
        sys.path.insert(0, _p)

import concourse.mybir as mybir  # noqa: E402
import concourse.tile as tile  # noqa: E402
from concourse import bacc  # noqa: E402
from concourse.bass_utils import run_bass_kernel_spmd  # noqa: E402

NCORES = 8
HID = 4096
NH = 32
NKV = 8
HD = 128
B = 4
S = 1024
T = B * S
GQ = NH // NCORES          # q heads per core = 4
DQ = GQ * HD               # 512
KT32 = HID // 128          # 32 k tiles
QCH = 512                  # token chunk
NQCH = T // QCH            # 8
SCALE = 1.0 / math.sqrt(HD)

FP32 = mybir.dt.float32
BF16 = mybir.dt.bfloat16
NP_BF16 = mybir.dt.np(BF16)

_PROG_CACHE: dict = {}


def _llama31_freqs_np(head_dim: int) -> np.ndarray:
    half = head_dim // 2
    theta, scale, low_ff, high_ff, old_ctx = 500000.0, 8.0, 1.0, 4.0, 8192.0
    freq = 1.0 / (theta ** (np.arange(half, dtype=np.float64) * 2.0 / head_dim))
    wavelen = 2.0 * np.pi / freq
    low_wl, high_wl = old_ctx / low_ff, old_ctx / high_ff
    smooth = (old_ctx / wavelen - low_ff) / (high_ff - low_ff)
    out = np.where(
        wavelen < high_wl,
        freq,
        np.where(wavelen > low_wl, freq / scale, (1.0 - smooth) * freq / scale + smooth * freq),
    )
    return out.astype(np.float64)


def _rope_tables(pos: np.ndarray) -> tuple[np.ndarray, np.ndarray]:
    """cosF [128, n]: cos duplicated on both partition halves.
    sinF2 [128, n]: +sin on rows 0-63, -sin on rows 64-127. The kernel
    computes out = x*cosF + halfswap(x*sinF2), which equals rotate-half
    RoPE."""
    freqs = _llama31_freqs_np(HD)
    ang = pos.astype(np.float64)[None, :] * freqs[:, None]  # [64, n]
    c = np.cos(ang).astype(np.float32)
    s = np.sin(ang).astype(np.float32)
    cosF = np.concatenate([c, c], axis=0)
    sinF2 = np.concatenate([s, -s], axis=0)
    return np.ascontiguousarray(cosF), np.ascontiguousarray(sinF2)


def _build_program(split_kv: bool):
    nc = bacc.Bacc(
        "TRN2",
        target_bir_lowering=False,
        debug=False,
        enable_asserts=False,
        num_devices=NCORES,
    )
    hT = nc.dram_tensor("hT", [HID, T], BF16, kind="ExternalInput")
    hTkv = (
        nc.dram_tensor("hTkv", [HID, T], BF16, kind="ExternalInput") if split_kv else hT
    )
    wqT = nc.dram_tensor("wqT", [HID, DQ], BF16, kind="ExternalInput")
    wkT = nc.dram_tensor("wkT", [HID, HD], BF16, kind="ExternalInput")
    wvT = nc.dram_tensor("wvT", [HID, HD], BF16, kind="ExternalInput")
    woT = nc.dram_tensor("woT", [DQ, HID], BF16, kind="ExternalInput")
    # K (page-rank order) positions are always 0..S-1 per sequence; a
    # [128, S] table sliced modulo S covers both passes in the identity
    # case and the KV pass in the permuted case.
    coskv = nc.dram_tensor("coskv", [128, S], FP32, kind="ExternalInput")
    sinkv = nc.dram_tensor("sinkv", [128, S], FP32, kind="ExternalInput")
    if split_kv:
        cosq = nc.dram_tensor("cosq", [128, T], FP32, kind="ExternalInput")
        sinq = nc.dram_tensor("sinq", [128, T], FP32, kind="ExternalInput")
    trid = nc.dram_tensor("trid", [128, 128], BF16, kind="ExternalInput")
    onesd = nc.dram_tensor("onesd", [128, 128], BF16, kind="ExternalInput")
    identd = nc.dram_tensor("identd", [128, 128], BF16, kind="ExternalInput")
    outp = nc.dram_tensor("outp", [T, HID], BF16, kind="ExternalOutput")

    with tile.TileContext(nc) as tc, ExitStack() as ctx:
        const_pool = ctx.enter_context(tc.tile_pool(name="const", bufs=1))
        QT = const_pool.tile([128, GQ, T], BF16)        # 32KB/part
        KT = const_pool.tile([128, T], BF16)            # 8KB
        V = const_pool.tile([128, T // 128, HD], BF16)  # 8KB (token-major tiles)
        aoT = const_pool.tile([128, GQ, T], BF16)       # 32KB
        tri_sb = const_pool.tile([128, 128], BF16)
        ones_sb = const_pool.tile([128, 128], BF16)
        ident_sb = const_pool.tile([128, 128], BF16)
        coskv_sb = const_pool.tile([128, S], FP32)
        sinkv_sb = const_pool.tile([128, S], FP32)
        # All DMAs stay on the single sync HWDGE queue: Tile assigns DMA
        # completion-sem lanes round-robin across queues while FIFO order
        # only holds per queue, so cross-queue DMAs can satisfy a
        # consumer's lane-wait out of order (observed as a flaky stale
        # weight read). Latency is managed by emission order instead:
        # constants are emitted inside the first chunk, below.
        if split_kv:
            cosq_sb = const_pool.tile([128, T], FP32)
            sinq_sb = const_pool.tile([128, T], FP32)
        else:
            cosq_sb, sinq_sb = coskv_sb, sinkv_sb

        def emit_const_dmas():
            nc.sync.dma_start(tri_sb[:], trid.ap()[:, :])
            nc.sync.dma_start(ones_sb[:], onesd.ap()[:, :])
            nc.sync.dma_start(ident_sb[:], identd.ap()[:, :])
            nc.sync.dma_start(coskv_sb[:], coskv.ap()[:, :])
            nc.sync.dma_start(sinkv_sb[:], sinkv.ap()[:, :])
            if split_kv:
                nc.sync.dma_start(cosq_sb[:], cosq.ap()[:, :])
                nc.sync.dma_start(sinq_sb[:], sinq.ap()[:, :])

        def rope_out(ps, cos_a, sin_a, out_full, tpool, n):
            """out = ps*cos + halfswap(ps*sinF2). The half-swap is two
            partition-shifted SBUF->SBUF DMAs (TensorTensor operands must
            share a start partition)."""
            t1 = tpool.tile([128, n], FP32, tag="t1", name="t1")
            u = tpool.tile([128, n], FP32, tag="u", name="u")
            u2 = tpool.tile([128, n], FP32, tag="u2", name="u2")
            nc.vector.tensor_mul(t1[:], ps[:], cos_a)
            nc.vector.tensor_mul(u[:], ps[:], sin_a)
            nc.sync.dma_start(u2[0:64, :], u[64:128, :])
            nc.sync.dma_start(u2[64:128, :], u[0:64, :])
            nc.vector.tensor_add(out_full, t1[:], u2[:])

        # Q pass runs FIRST (its 4-rope chunk evacuation is the expensive
        # one; the KV pass's cheaper evacuation then gates phase 2's PSUM
        # handoff). The h pool is shared by both passes so the second
        # pass's h DMAs start while the first pass is still computing.
        h_r = hT.ap().rearrange("(ko p) t -> p ko t", p=128)
        hkv_r = hTkv.ap().rearrange("(ko p) t -> p ko t", p=128)
        wq_r = wqT.ap().rearrange("(ko p) d -> p ko d", p=128)
        wk_r = wkT.ap().rearrange("(ko p) d -> p ko d", p=128)
        wv_r = wvT.ap().rearrange("(ko p) d -> p ko d", p=128)
        with tc.tile_pool(name="wqkv", bufs=1) as wpool, \
             tc.tile_pool(name="h1", bufs=5 if not split_kv else 4) as hpool:
            wq_sb = wpool.tile([128, KT32, DQ], BF16)
            wk_sb = wpool.tile([128, KT32, HD], BF16)
            wv_sb = wpool.tile([128, KT32, HD], BF16)
            # Startup streaming: wq k-groups are interleaved with chunk
            # 0's h tiles in sync-queue FIFO order, so the first matmuls
            # start after ~2MB instead of after all weights. Constants
            # land after chunk 0's h stream (needed at the first rope),
            # wk/wv after chunk 1's (needed a whole pass later).
            nc.sync.dma_start(wq_sb[:, 0:4, :], wq_r[:, 0:4, :])

            # ------------- phase 1b: Q projections -----------------------
            with tc.tile_pool(name="ps1b", bufs=2, space="PSUM") as ppool, \
                 tc.tile_pool(name="st1b", bufs=2 if not split_kv else 1) as stpool:
                for c in range(NQCH):
                    tsl = slice(c * QCH, (c + 1) * QCH)
                    # Chunk 0 streams at finer granularity, alternating wq
                    # k-groups with h pieces so the first matmuls start as
                    # early as possible.
                    kper = 4 if c == 0 else 8
                    hts = []
                    for j in range(32 // kper):
                        ht = hpool.tile([128, kper, QCH], BF16, tag="h", name="h")
                        nc.sync.dma_start(
                            ht[:], h_r[:, j * kper:(j + 1) * kper, tsl]
                        )
                        hts.append(ht)
                        if c == 0 and j + 1 < 8:
                            nc.sync.dma_start(
                                wq_sb[:, 4 * (j + 1):4 * (j + 1) + 4, :],
                                wq_r[:, 4 * (j + 1):4 * (j + 1) + 4, :],
                            )
                    if c == 0:
                        emit_const_dmas()
                    elif c == 1:
                        for kg in range(0, KT32, 8):
                            nc.sync.dma_start(
                                wk_sb[:, kg:kg + 8, :], wk_r[:, kg:kg + 8, :]
                            )
                            nc.sync.dma_start(
                                wv_sb[:, kg:kg + 8, :], wv_r[:, kg:kg + 8, :]
                            )
                    if split_kv:
                        cs, ss = cosq_sb[:, tsl], sinq_sb[:, tsl]
                    else:
                        p0 = (c * QCH) % S
                        cs, ss = cosq_sb[:, p0:p0 + QCH], sinq_sb[:, p0:p0 + QCH]
                    # The last chunk runs as two 2-head sub-passes so its
                    # final rope evacuation overlaps the second sub-pass's
                    # matmuls instead of stalling the next phase's PSUM
                    # handoff.
                    groups = (
                        [tuple(range(GQ))] if c < NQCH - 1 else [(0, 1), (2, 3)]
                    )
                    if c == 0:
                        # Preload the scalar engine's Exp spline tables off
                        # the critical path (first use otherwise stalls the
                        # attention phase ~2.7us).
                        warm = stpool.tile([128, 1], FP32, tag="warm", name="warm")
                        nc.scalar.activation(
                            warm[:], coskv_sb[:, 0:1],
                            mybir.ActivationFunctionType.Exp,
                        )
                    for gs in groups:
                        psq = {
                            g: ppool.tile(
                                [128, QCH], FP32, tag=f"psq{g}", name=f"psq{g}"
                            )
                            for g in gs
                        }
                        for k in range(KT32):
                            ht = hts[k // kper][:, k % kper, :]
                            st = k == 0
                            sp = k == KT32 - 1
                            for g in gs:
                                nc.tensor.matmul(
                                    psq[g][:], wq_sb[:, k, g * 128:(g + 1) * 128],
                                    ht, start=st, stop=sp,
                                )
                        for g in gs:
                            rope_out(psq[g], cs, ss, QT[:, g, tsl], stpool, QCH)

            # ------------- phase 1a: K/V projections ---------------------
            with tc.tile_pool(name="ps1a", bufs=2, space="PSUM") as ppool, \
                 tc.tile_pool(name="st1a", bufs=2 if not split_kv else 1) as stpool:
                for c in range(NQCH):
                    tsl = slice(c * QCH, (c + 1) * QCH)
                    hts = []
                    for j in range(4):
                        ht = hpool.tile([128, 8, QCH], BF16, tag="h", name="h")
                        nc.sync.dma_start(ht[:], hkv_r[:, j * 8:(j + 1) * 8, tsl])
                        hts.append(ht)
                    p0 = (c * QCH) % S

                    def emit_k(psk):
                        rope_out(
                            psk, coskv_sb[:, p0:p0 + QCH], sinkv_sb[:, p0:p0 + QCH],
                            KT[:, tsl], stpool, QCH,
                        )

                    def emit_v(psv):
                        vsb = stpool.tile([128, QCH], BF16, tag="vsb", name="vsb")
                        nc.scalar.copy(vsb[:], psv[:])
                        pst = ppool.tile([128, 4, 128], BF16, tag="pst", name="pst")
                        for i in range(4):
                            nc.tensor.transpose(
                                pst[:, i, :], vsb[:, i * 128:(i + 1) * 128],
                                ident_sb[:],
                            )
                            nc.vector.tensor_copy(V[:, 4 * c + i, :], pst[:, i, :])

                    if c < NQCH - 1:
                        psk = ppool.tile([128, QCH], FP32, tag="psk", name="psk")
                        psv = ppool.tile([128, QCH], FP32, tag="psv", name="psv")
                        for k in range(KT32):
                            ht = hts[k // 8][:, k % 8, :]
                            st = k == 0
                            sp = k == KT32 - 1
                            nc.tensor.matmul(
                                psk[:], wk_sb[:, k, :], ht, start=st, stop=sp
                            )
                            nc.tensor.matmul(
                                psv[:], wv_sb[:, k, :], ht, start=st, stop=sp
                            )
                        emit_k(psk)
                        emit_v(psv)
                    else:
                        # Last chunk: K sweep then V sweep, so the K rope
                        # overlaps the V matmuls and only the short V
                        # evacuation gates the attention phase.
                        psk = ppool.tile([128, QCH], FP32, tag="psk", name="psk")
                        for k in range(KT32):
                            nc.tensor.matmul(
                                psk[:], wk_sb[:, k, :], hts[k // 8][:, k % 8, :],
                                start=(k == 0), stop=(k == KT32 - 1),
                            )
                        emit_k(psk)
                        psv = ppool.tile([128, QCH], FP32, tag="psv", name="psv")
                        for k in range(KT32):
                            nc.tensor.matmul(
                                psv[:], wv_sb[:, k, :], hts[k // 8][:, k % 8, :],
                                start=(k == 0), stop=(k == KT32 - 1),
                            )
                        emit_v(psv)

        # ------------- phases 2+3, interleaved per batch ------------------
        wo_r = woT.ap().rearrange("(g p) e -> p g e", p=128)
        with tc.tile_pool(name="wo", bufs=1) as wopool:
            wo_sb = wopool.tile([128, GQ, HID], BF16)
            for g in range(GQ):
                nc.sync.dma_start(wo_sb[:, g, :], wo_r[:, g, :])

            with tc.tile_pool(name="sb2", bufs=2) as sbpool, \
                 tc.tile_pool(name="ex2", bufs=10) as expool, \
                 tc.tile_pool(name="pss2", bufs=3, space="PSUM") as pspool, \
                 tc.tile_pool(name="pv2", bufs=2, space="PSUM") as pvpool, \
                 tc.tile_pool(name="psd2", bufs=1, space="PSUM") as pdpool, \
                 tc.tile_pool(name="ps3", bufs=2, space="PSUM") as p3pool, \
                 tc.tile_pool(name="ob3", bufs=3) as obpool:
                def p2_unit(b, g, it):
                    qoff = it * QCH
                    q0 = b * S + qoff
                    njt = (qoff + QCH) // 128
                    es = sbpool.tile([128, QCH], BF16, tag="es", name="es")
                    pv = pvpool.tile([128, QCH], FP32, tag="pv", name="pv")
                    exs, offs = [], []
                    # All score matmuls stream first (the scalar engine's
                    # exps run one behind), then all PV matmuls — the PE
                    # never waits on the exp chain.
                    for jt in range(njt):
                        ko = b * S + jt * 128
                        off = jt * 128 - qoff if jt * 128 >= qoff else 0
                        pss = pspool.tile([128, QCH], FP32, tag="pss", name="pss")
                        nc.tensor.matmul(
                            pss[:, off:QCH],
                            KT[:, ko:ko + 128],
                            QT[:, g, q0 + off:q0 + QCH],
                            start=True, stop=True,
                        )
                        ex = expool.tile([128, QCH], BF16, tag="ex", name="ex")
                        nc.scalar.activation(
                            ex[:, off:QCH], pss[:, off:QCH],
                            mybir.ActivationFunctionType.Exp, scale=SCALE,
                        )
                        if jt * 128 >= qoff:  # diagonal block
                            nc.vector.tensor_mul(
                                ex[:, off:off + 128], ex[:, off:off + 128],
                                tri_sb[:],
                            )
                        if jt == 0:
                            nc.vector.tensor_copy(es[:], ex[:])
                        else:
                            nc.vector.tensor_add(
                                es[:, off:QCH], es[:, off:QCH], ex[:, off:QCH]
                            )
                        exs.append(ex)
                        offs.append(off)
                    for jt in range(njt):
                        nc.tensor.matmul(
                            pv[:, offs[jt]:QCH],
                            V[:, b * 8 + jt, :],
                            exs[jt][:, offs[jt]:QCH],
                            start=(jt == 0), stop=(jt == njt - 1),
                        )
                    psd = pdpool.tile([128, QCH], FP32, tag="psd", name="psd")
                    nc.tensor.matmul(psd[:], ones_sb[:], es[:], start=True, stop=True)
                    rec = sbpool.tile([128, QCH], FP32, tag="rec", name="rec")
                    nc.vector.reciprocal_approx_fast(rec[:], psd[:])
                    nc.vector.tensor_mul(aoT[:, g, q0:q0 + QCH], pv[:], rec[:])

                def p3_unit(eg, tb, fine=False):
                    ob = obpool.tile([128, 4, QCH], BF16, tag="ob", name="ob")
                    for ei in range(4):
                        e0 = eg * 2048 + ei * QCH
                        pso = p3pool.tile([128, QCH], FP32, tag="pso", name="pso")
                        for g in range(GQ):
                            nc.tensor.matmul(
                                pso[:],
                                aoT[:, g, tb * 128:(tb + 1) * 128],
                                wo_sb[:, g, e0:e0 + QCH],
                                start=(g == 0), stop=(g == GQ - 1),
                            )
                        if ei % 2 == 0:
                            nc.scalar.copy(ob[:, ei, :], pso[:])
                        else:
                            nc.vector.tensor_copy(ob[:, ei, :], pso[:])
                        if fine:
                            # Tail units: ship each 512-col piece as soon as
                            # it is evacuated so the final DMA is small.
                            nc.sync.dma_start(
                                outp.ap()[tb * 128:(tb + 1) * 128, e0:e0 + QCH],
                                ob[:, ei, :],
                            )
                        elif ei == 1:
                            nc.sync.dma_start(
                                outp.ap()[tb * 128:(tb + 1) * 128,
                                          eg * 2048:eg * 2048 + 1024],
                                ob[:, 0:2, :],
                            )
                    if not fine:
                        nc.sync.dma_start(
                            outp.ap()[tb * 128:(tb + 1) * 128,
                                      eg * 2048 + 1024:(eg + 1) * 2048],
                            ob[:, 2:4, :],
                        )

                # Batch b's attention units are interleaved with batch
                # b-1's output-projection units: the projection's dense
                # matmul stream fills the PE whenever attention waits on
                # the exp chain.
                for b in range(B):
                    fill = (
                        [(eg, tb) for eg in range(2)
                         for tb in range(8 * (b - 1), 8 * (b - 1) + 8)]
                        if b > 0 else []
                    )
                    ui = 0
                    for g in range(GQ):
                        for it in range(2):
                            p2_unit(b, g, it)
                            for eg, tb in fill[2 * ui:2 * ui + 2]:
                                p3_unit(eg, tb)
                            ui += 1
                tail_units = [
                    (eg, tb) for eg in range(2)
                    for tb in range(8 * (B - 1), 8 * (B - 1) + 8)
                ]
                for i, (eg, tb) in enumerate(tail_units):
                    p3_unit(eg, tb, fine=(i >= len(tail_units) - 2))

    nc.finalize()
    return nc


def _get_program(split_kv: bool):
    if split_kv not in _PROG_CACHE:
        _PROG_CACHE[split_kv] = _build_program(split_kv)
    return _PROG_CACHE[split_kv]


def kernel(
    hidden_states, wq, wk, wv, wo, kv_cache, position_ids,
    kv_page_indices, kv_page_indptr, kv_last_page_lens, qo_indptr,
    _run_kwargs: dict | None = None,
):
    hidden_states = np.asarray(hidden_states, np.float32)
    wq = np.asarray(wq, np.float32)
    wk = np.asarray(wk, np.float32)
    wv = np.asarray(wv, np.float32)
    wo = np.asarray(wo, np.float32)
    position_ids = np.asarray(position_ids, np.int32)
    qo_indptr = np.asarray(qo_indptr, np.int64)

    nnz = hidden_states.shape[0]
    b = qo_indptr.shape[0] - 1
    assert nnz == T and b == B, (nnz, b)
    assert np.array_equal(qo_indptr, np.arange(B + 1, dtype=np.int64) * S), (
        "kernel assumes uniform sequence lengths of 1024"
    )

    # Page-gather order: the reference gathers pages in list order, so the
    # token with position p within its sequence lands at page-order rank p.
    # KV must be fed in rank order; the q path stays in token order.
    perm = np.empty(T, np.int64)
    identity = True
    for bi in range(B):
        pos_b = position_ids[bi * S:(bi + 1) * S].astype(np.int64)
        assert np.array_equal(np.sort(pos_b), np.arange(S)), (
            "kernel assumes positions cover 0..S-1 exactly once per sequence"
        )
        inv = np.empty(S, np.int64)
        inv[pos_b] = np.arange(S)
        perm[bi * S:(bi + 1) * S] = bi * S + inv
        if not np.array_equal(inv, np.arange(S)):
            identity = False

    hT16 = np.ascontiguousarray(hidden_states.T.astype(NP_BF16))
    coskv, sinkv = _rope_tables(np.arange(S, dtype=np.int64))
    tri = np.ascontiguousarray(
        (np.arange(128)[:, None] <= np.arange(128)[None, :]).astype(NP_BF16)
    )
    ones = np.ones((128, 128), NP_BF16)
    eye = np.eye(128, dtype=np.float32).astype(NP_BF16)

    split_kv = not identity
    nc = _get_program(split_kv)

    in_maps = []
    for c in range(NCORES):
        im = {
            "hT": hT16,
            "wqT": np.ascontiguousarray(wq[c * DQ:(c + 1) * DQ, :].T.astype(NP_BF16)),
            "wkT": np.ascontiguousarray(wk[c * HD:(c + 1) * HD, :].T.astype(NP_BF16)),
            "wvT": np.ascontiguousarray(wv[c * HD:(c + 1) * HD, :].T.astype(NP_BF16)),
            "woT": np.ascontiguousarray(wo[:, c * DQ:(c + 1) * DQ].T.astype(NP_BF16)),
            "coskv": coskv,
            "sinkv": sinkv,
            "trid": tri,
            "onesd": ones,
            "identd": eye,
        }
        if split_kv:
            im["hTkv"] = np.ascontiguousarray(hT16[:, perm])
            cosq, sinq = _rope_tables(position_ids)
            im["cosq"] = cosq
            im["sinq"] = sinq
        in_maps.append(im)

    res = run_bass_kernel_spmd(
        nc, in_maps, core_ids=list(range(NCORES)), **(_run_kwargs or {})
    )
    out = np.zeros((T, HID), np.float32)
    for c in range(NCORES):
        out += res.results[c]["outp"].astype(np.float32)
    kernel.last_results = res  # type: ignore[attr-defined]
    return out


# revision 30
# speedup vs baseline: 1.1421x; 1.0099x over previous
"""Trainium2 Bass kernel for paged-attention Llama-style block (nn_L4maAttention).

Sharding: tensor-parallel over heads across 8 NeuronCores. Core c owns
q-heads [4c, 4c+4), kv-head c, wq/wk/wv row shards and the matching wo
column shard. Each core computes a full [T, HID] partial of the output
projection in bf16; the host sums the 8 partials (the TP reduce).

Device kernel (per core), matmuls in bf16 (full PE rate, half the DMA
of fp32), fp32 PSUM accumulation. All DMAs stay on the single sync
HWDGE queue (see the in-code note on cross-queue completion-sem
ordering); startup latency is managed by emission order instead.

  phase 1b (Q pass, first): Q projections in 8 token chunks of 512,
        4 heads x [128,512] PSUM (4 banks x 2 buffers = all 8), RoPE
        -> QT (SBUF resident, bf16). RoPE computes x*cos + swap*sinF2
        with the half-swap done by two partition-shifted SBUF->SBUF
        DMAs. Chunk 0 interleaves wq k-group DMAs with its h pieces so
        the first matmul starts after ~1MB; the exp activation table
        is preloaded here. The last chunk runs as two 2-head
        sub-passes so its rope evacuation overlaps matmuls.
  phase 1a (KV pass): K/V projections, same chunking, PSUM psk + psv
        + transpose bank, x2 buffers = 6 banks. K^T + RoPE -> KT;
        V computed head-dim-major then PE-transposed (identity
        matmul) to token-major tiles in V. Last chunk splits into a
        K sweep then V sweep so only the short V evacuation gates the
        attention phase's PSUM handoff.
  phases 2+3, interleaved per batch: attention unit (b,g,it) =
        transposed scores [k on partitions, q free] over the causal
        suffix of each 128-row k block; exp in bf16 on the scalar
        engine with 1/sqrt(d) folded into the activation scale;
        single [128,128] triangular mask on the diagonal sub-block;
        all score matmuls stream first, then all PV matmuls, so the
        PE never waits on the exp chain; denominators via a
        ones-matmul (broadcast over partitions) + fast reciprocal.
        Batch b-1's output-projection units (aoT.T @ woT, evacuation
        alternating scalar/vector, split 256KB output DMAs) are
        emitted between batch b's attention units as dense PE filler.
"""

import math
import sys
from contextlib import ExitStack

import numpy as np

for _p in ("/opt/trn_rl_repo",):
    if _p not in sys.path:
        sys.path.insert(0, _p)

import concourse.mybir as mybir  # noqa: E402
import concourse.tile as tile  # noqa: E402
from concourse import bacc  # noqa: E402
from concourse.bass_utils import run_bass_kernel_spmd  # noqa: E402

NCORES = 8
HID = 4096
NH = 32
NKV = 8
HD = 128
B = 4
S = 1024
T = B * S
GQ = NH // NCORES          # q heads per core = 4
DQ = GQ * HD               # 512
KT32 = HID // 128          # 32 k tiles
QCH = 512                  # token chunk
NQCH = T // QCH            # 8
SCALE = 1.0 / math.sqrt(HD)

FP32 = mybir.dt.float32
BF16 = mybir.dt.bfloat16
NP_BF16 = mybir.dt.np(BF16)

_PROG_CACHE: dict = {}


def _llama31_freqs_np(head_dim: int) -> np.ndarray:
    half = head_dim // 2
    theta, scale, low_ff, high_ff, old_ctx = 500000.0, 8.0, 1.0, 4.0, 8192.0
    freq = 1.0 / (theta ** (np.arange(half, dtype=np.float64) * 2.0 / head_dim))
    wavelen = 2.0 * np.pi / freq
    low_wl, high_wl = old_ctx / low_ff, old_ctx / high_ff
    smooth = (old_ctx / wavelen - low_ff) / (high_ff - low_ff)
    out = np.where(
        wavelen < high_wl,
        freq,
        np.where(wavelen > low_wl, freq / scale, (1.0 - smooth) * freq / scale + smooth * freq),
    )
    return out.astype(np.float64)


def _rope_tables(pos: np.ndarray) -> tuple[np.ndarray, np.ndarray]:
    """cosF [128, n]: cos duplicated on both partition halves.
    sinF2 [128, n]: +sin on rows 0-63, -sin on rows 64-127. The kernel
    computes out = x*cosF + halfswap(x*sinF2), which equals rotate-half
    RoPE."""
    freqs = _llama31_freqs_np(HD)
    ang = pos.astype(np.float64)[None, :] * freqs[:, None]  # [64, n]
    c = np.cos(ang).astype(np.float32)
    s = np.sin(ang).astype(np.float32)
    cosF = np.concatenate([c, c], axis=0)
    sinF2 = np.concatenate([s, -s], axis=0)
    return np.ascontiguousarray(cosF), np.ascontiguousarray(sinF2)


def _build_program(split_kv: bool):
    nc = bacc.Bacc(
        "TRN2",
        target_bir_lowering=False,
        debug=False,
        enable_asserts=False,
        num_devices=NCORES,
    )
    hT = nc.dram_tensor("hT", [HID, T], BF16, kind="ExternalInput")
    hTkv = (
        nc.dram_tensor("hTkv", [HID, T], BF16, kind="ExternalInput") if split_kv else hT
    )
    wqT = nc.dram_tensor("wqT", [HID, DQ], BF16, kind="ExternalInput")
    wkT = nc.dram_tensor("wkT", [HID, HD], BF16, kind="ExternalInput")
    wvT = nc.dram_tensor("wvT", [HID, HD], BF16, kind="ExternalInput")
    woT = nc.dram_tensor("woT", [DQ, HID], BF16, kind="ExternalInput")
    # K (page-rank order) positions are always 0..S-1 per sequence; a
    # [128, S] table sliced modulo S covers both passes in the identity
    # case and the KV pass in the permuted case.
    coskv = nc.dram_tensor("coskv", [128, S], FP32, kind="ExternalInput")
    sinkv = nc.dram_tensor("sinkv", [128, S], FP32, kind="ExternalInput")
    if split_kv:
        cosq = nc.dram_tensor("cosq", [128, T], FP32, kind="ExternalInput")
        sinq = nc.dram_tensor("sinq", [128, T], FP32, kind="ExternalInput")
    trid = nc.dram_tensor("trid", [128, 128], BF16, kind="ExternalInput")
    onesd = nc.dram_tensor("onesd", [128, 128], BF16, kind="ExternalInput")
    identd = nc.dram_tensor("identd", [128, 128], BF16, kind="ExternalInput")
    outp = nc.dram_tensor("outp", [T, HID], BF16, kind="ExternalOutput")

    with tile.TileContext(nc) as tc, ExitStack() as ctx:
        const_pool = ctx.enter_context(tc.tile_pool(name="const", bufs=1))
        QT = const_pool.tile([128, GQ, T], BF16)        # 32KB/part
        KT = const_pool.tile([128, T], BF16)            # 8KB
        V = const_pool.tile([128, T // 128, HD], BF16)  # 8KB (token-major tiles)
        aoT = const_pool.tile([128, GQ, T], BF16)       # 32KB
        tri_sb = const_pool.tile([128, 128], BF16)
        ones_sb = const_pool.tile([128, 128], BF16)
        ident_sb = const_pool.tile([128, 128], BF16)
        coskv_sb = const_pool.tile([128, S], FP32)
        sinkv_sb = const_pool.tile([128, S], FP32)
        # All DMAs stay on the single sync HWDGE queue: Tile assigns DMA
        # completion-sem lanes round-robin across queues while FIFO order
        # only holds per queue, so cross-queue DMAs can satisfy a
        # consumer's lane-wait out of order (observed as a flaky stale
        # weight read). Latency is managed by emission order instead:
        # constants are emitted inside the first chunk, below.
        if split_kv:
            cosq_sb = const_pool.tile([128, T], FP32)
            sinq_sb = const_pool.tile([128, T], FP32)
        else:
            cosq_sb, sinq_sb = coskv_sb, sinkv_sb

        def emit_const_dmas():
            nc.sync.dma_start(tri_sb[:], trid.ap()[:, :])
            nc.sync.dma_start(ones_sb[:], onesd.ap()[:, :])
            nc.sync.dma_start(ident_sb[:], identd.ap()[:, :])
            nc.sync.dma_start(coskv_sb[:], coskv.ap()[:, :])
            nc.sync.dma_start(sinkv_sb[:], sinkv.ap()[:, :])
            if split_kv:
                nc.sync.dma_start(cosq_sb[:], cosq.ap()[:, :])
                nc.sync.dma_start(sinq_sb[:], sinq.ap()[:, :])

        def rope_out(ps, cos_a, sin_a, out_full, tpool, n):
            """out = ps*cos + halfswap(ps*sinF2). The half-swap is two
            partition-shifted SBUF->SBUF DMAs (TensorTensor operands must
            share a start partition)."""
            t1 = tpool.tile([128, n], FP32, tag="t1", name="t1")
            u = tpool.tile([128, n], FP32, tag="u", name="u")
            u2 = tpool.tile([128, n], FP32, tag="u2", name="u2")
            nc.vector.tensor_mul(t1[:], ps[:], cos_a)
            nc.vector.tensor_mul(u[:], ps[:], sin_a)
            nc.sync.dma_start(u2[0:64, :], u[64:128, :])
            nc.sync.dma_start(u2[64:128, :], u[0:64, :])
            nc.vector.tensor_add(out_full, t1[:], u2[:])

        # Q pass runs FIRST (its 4-rope chunk evacuation is the expensive
        # one; the KV pass's cheaper evacuation then gates phase 2's PSUM
        # handoff). The h pool is shared by both passes so the second
        # pass's h DMAs start while the first pass is still computing.
        h_r = hT.ap().rearrange("(ko p) t -> p ko t", p=128)
        hkv_r = hTkv.ap().rearrange("(ko p) t -> p ko t", p=128)
        wq_r = wqT.ap().rearrange("(ko p) d -> p ko d", p=128)
        wk_r = wkT.ap().rearrange("(ko p) d -> p ko d", p=128)
        wv_r = wvT.ap().rearrange("(ko p) d -> p ko d", p=128)
        with tc.tile_pool(name="wqkv", bufs=1) as wpool, \
             tc.tile_pool(name="h1", bufs=5 if not split_kv else 4) as hpool:
            wq_sb = wpool.tile([128, KT32, DQ], BF16)
            wk_sb = wpool.tile([128, KT32, HD], BF16)
            wv_sb = wpool.tile([128, KT32, HD], BF16)
            # Startup streaming: wq k-groups are interleaved with chunk
            # 0's h tiles in sync-queue FIFO order, so the first matmuls
            # start after ~2MB instead of after all weights. Constants
            # land after chunk 0's h stream (needed at the first rope),
            # wk/wv after chunk 1's (needed a whole pass later).
            nc.sync.dma_start(wq_sb[:, 0:4, :], wq_r[:, 0:4, :])

            # ------------- phase 1b: Q projections -----------------------
            with tc.tile_pool(name="ps1b", bufs=2, space="PSUM") as ppool, \
                 tc.tile_pool(name="st1b", bufs=2 if not split_kv else 1) as stpool:
                for c in range(NQCH):
                    tsl = slice(c * QCH, (c + 1) * QCH)
                    # Chunk 0 streams at finer granularity, alternating wq
                    # k-groups with h pieces so the first matmuls start as
                    # early as possible.
                    kper = 4 if c == 0 else 8
                    hts = []
                    for j in range(32 // kper):
                        ht = hpool.tile([128, kper, QCH], BF16, tag="h", name="h")
                        nc.sync.dma_start(
                            ht[:], h_r[:, j * kper:(j + 1) * kper, tsl]
                        )
                        hts.append(ht)
                        if c == 0 and j + 1 < 8:
                            nc.sync.dma_start(
                                wq_sb[:, 4 * (j + 1):4 * (j + 1) + 4, :],
                                wq_r[:, 4 * (j + 1):4 * (j + 1) + 4, :],
                            )
                    if c == 0:
                        emit_const_dmas()
                    elif c == 3:
                        for kg in range(0, KT32, 8):
                            nc.sync.dma_start(
                                wk_sb[:, kg:kg + 8, :], wk_r[:, kg:kg + 8, :]
                            )
                            nc.sync.dma_start(
                                wv_sb[:, kg:kg + 8, :], wv_r[:, kg:kg + 8, :]
                            )
                    if split_kv:
                        cs, ss = cosq_sb[:, tsl], sinq_sb[:, tsl]
                    else:
                        p0 = (c * QCH) % S
                        cs, ss = cosq_sb[:, p0:p0 + QCH], sinq_sb[:, p0:p0 + QCH]
                    # The last chunk runs as two 2-head sub-passes so its
                    # final rope evacuation overlaps the second sub-pass's
                    # matmuls instead of stalling the next phase's PSUM
                    # handoff.
                    groups = (
                        [tuple(range(GQ))] if c < NQCH - 1
                        else [(0, 1), (2,), (3,)]
                    )
                    if c == 0:
                        # Preload the scalar engine's Exp spline tables off
                        # the critical path (first use otherwise stalls the
                        # attention phase ~2.7us).
                        warm = stpool.tile([128, 1], FP32, tag="warm", name="warm")
                        nc.scalar.activation(
                            warm[:], coskv_sb[:, 0:1],
                            mybir.ActivationFunctionType.Exp,
                        )
                    for gs in groups:
                        psq = {
                            g: ppool.tile(
                                [128, QCH], FP32, tag=f"psq{g}", name=f"psq{g}"
                            )
                            for g in gs
                        }
                        for k in range(KT32):
                            ht = hts[k // kper][:, k % kper, :]
                            st = k == 0
                            sp = k == KT32 - 1
                            for g in gs:
                                nc.tensor.matmul(
                                    psq[g][:], wq_sb[:, k, g * 128:(g + 1) * 128],
                                    ht, start=st, stop=sp,
                                )
                        for g in gs:
                            rope_out(psq[g], cs, ss, QT[:, g, tsl], stpool, QCH)

            # ------------- phase 1a: K/V projections ---------------------
            with tc.tile_pool(name="ps1a", bufs=2, space="PSUM") as ppool, \
                 tc.tile_pool(name="st1a", bufs=2 if not split_kv else 1) as stpool:
                for c in range(NQCH):
                    tsl = slice(c * QCH, (c + 1) * QCH)
                    hts = []
                    for j in range(4):
                        ht = hpool.tile([128, 8, QCH], BF16, tag="h", name="h")
                        nc.sync.dma_start(ht[:], hkv_r[:, j * 8:(j + 1) * 8, tsl])
                        hts.append(ht)
                    p0 = (c * QCH) % S

                    def emit_k(psk):
                        rope_out(
                            psk, coskv_sb[:, p0:p0 + QCH], sinkv_sb[:, p0:p0 + QCH],
                            KT[:, tsl], stpool, QCH,
                        )

                    def emit_v(psv):
                        vsb = stpool.tile([128, QCH], BF16, tag="vsb", name="vsb")
                        nc.scalar.copy(vsb[:], psv[:])
                        pst = ppool.tile([128, 4, 128], BF16, tag="pst", name="pst")
                        for i in range(4):
                            nc.tensor.transpose(
                                pst[:, i, :], vsb[:, i * 128:(i + 1) * 128],
                                ident_sb[:],
                            )
                            nc.vector.tensor_copy(V[:, 4 * c + i, :], pst[:, i, :])

                    if c < NQCH - 1:
                        psk = ppool.tile([128, QCH], FP32, tag="psk", name="psk")
                        psv = ppool.tile([128, QCH], FP32, tag="psv", name="psv")
                        for k in range(KT32):
                            ht = hts[k // 8][:, k % 8, :]
                            st = k == 0
                            sp = k == KT32 - 1
                            nc.tensor.matmul(
                                psk[:], wk_sb[:, k, :], ht, start=st, stop=sp
                            )
                            nc.tensor.matmul(
                                psv[:], wv_sb[:, k, :], ht, start=st, stop=sp
                            )
                        emit_k(psk)
                        emit_v(psv)
                    else:
                        # Last chunk: K sweep then V sweep, so the K rope
                        # overlaps the V matmuls and only the short V
                        # evacuation gates the attention phase.
                        psk = ppool.tile([128, QCH], FP32, tag="psk", name="psk")
                        for k in range(KT32):
                            nc.tensor.matmul(
                                psk[:], wk_sb[:, k, :], hts[k // 8][:, k % 8, :],
                                start=(k == 0), stop=(k == KT32 - 1),
                            )
                        emit_k(psk)
                        psv = ppool.tile([128, QCH], FP32, tag="psv", name="psv")
                        for k in range(KT32):
                            nc.tensor.matmul(
                                psv[:], wv_sb[:, k, :], hts[k // 8][:, k % 8, :],
                                start=(k == 0), stop=(k == KT32 - 1),
                            )
                        emit_v(psv)

        # ------------- phases 2+3, interleaved per batch ------------------
        wo_r = woT.ap().rearrange("(g p) e -> p g e", p=128)
        with tc.tile_pool(name="wo", bufs=1) as wopool:
            wo_sb = wopool.tile([128, GQ, HID], BF16)
            for g in range(GQ):
                nc.sync.dma_start(wo_sb[:, g, :], wo_r[:, g, :])

            with tc.tile_pool(name="sb2", bufs=2) as sbpool, \
                 tc.tile_pool(name="ex2", bufs=10) as expool, \
                 tc.tile_pool(name="pss2", bufs=3, space="PSUM") as pspool, \
                 tc.tile_pool(name="pv2", bufs=2, space="PSUM") as pvpool, \
                 tc.tile_pool(name="psd2", bufs=1, space="PSUM") as pdpool, \
                 tc.tile_pool(name="ps3", bufs=2, space="PSUM") as p3pool, \
                 tc.tile_pool(name="ob3", bufs=3) as obpool:
                def p2_unit(b, g, it):
                    qoff = it * QCH
                    q0 = b * S + qoff
                    njt = (qoff + QCH) // 128
                    es = sbpool.tile([128, QCH], BF16, tag="es", name="es")
                    pv = pvpool.tile([128, QCH], FP32, tag="pv", name="pv")
                    exs, offs = [], []
                    # All score matmuls stream first (the scalar engine's
                    # exps run one behind), then all PV matmuls — the PE
                    # never waits on the exp chain.
                    for jt in range(njt):
                        ko = b * S + jt * 128
                        off = jt * 128 - qoff if jt * 128 >= qoff else 0
                        pss = pspool.tile([128, QCH], FP32, tag="pss", name="pss")
                        nc.tensor.matmul(
                            pss[:, off:QCH],
                            KT[:, ko:ko + 128],
                            QT[:, g, q0 + off:q0 + QCH],
                            start=True, stop=True,
                        )
                        ex = expool.tile([128, QCH], BF16, tag="ex", name="ex")
                        nc.scalar.activation(
                            ex[:, off:QCH], pss[:, off:QCH],
                            mybir.ActivationFunctionType.Exp, scale=SCALE,
                        )
                        if jt * 128 >= qoff:  # diagonal block
                            nc.vector.tensor_mul(
                                ex[:, off:off + 128], ex[:, off:off + 128],
                                tri_sb[:],
                            )
                        if jt == 0:
                            nc.vector.tensor_copy(es[:], ex[:])
                        else:
                            nc.vector.tensor_add(
                                es[:, off:QCH], es[:, off:QCH], ex[:, off:QCH]
                            )
                        exs.append(ex)
                        offs.append(off)
                    for jt in range(njt):
                        nc.tensor.matmul(
                            pv[:, offs[jt]:QCH],
                            V[:, b * 8 + jt, :],
                            exs[jt][:, offs[jt]:QCH],
                            start=(jt == 0), stop=(jt == njt - 1),
                        )
                    psd = pdpool.tile([128, QCH], FP32, tag="psd", name="psd")
                    nc.tensor.matmul(psd[:], ones_sb[:], es[:], start=True, stop=True)
                    rec = sbpool.tile([128, QCH], FP32, tag="rec", name="rec")
                    nc.vector.reciprocal_approx_fast(rec[:], psd[:])
                    nc.vector.tensor_mul(aoT[:, g, q0:q0 + QCH], pv[:], rec[:])

                def p3_unit(eg, tb, fine=False):
                    ob = obpool.tile([128, 4, QCH], BF16, tag="ob", name="ob")
                    for ei in range(4):
                        e0 = eg * 2048 + ei * QCH
                        pso = p3pool.tile([128, QCH], FP32, tag="pso", name="pso")
                        for g in range(GQ):
                            nc.tensor.matmul(
                                pso[:],
                                aoT[:, g, tb * 128:(tb + 1) * 128],
                                wo_sb[:, g, e0:e0 + QCH],
                                start=(g == 0), stop=(g == GQ - 1),
                            )
                        if ei % 2 == 0:
                            nc.scalar.copy(ob[:, ei, :], pso[:])
                        else:
                            nc.vector.tensor_copy(ob[:, ei, :], pso[:])
                        if fine:
                            # Tail units: ship each 512-col piece as soon as
                            # it is evacuated so the final DMA is small.
                            nc.sync.dma_start(
                                outp.ap()[tb * 128:(tb + 1) * 128, e0:e0 + QCH],
                                ob[:, ei, :],
                            )
                        elif ei == 1:
                            nc.sync.dma_start(
                                outp.ap()[tb * 128:(tb + 1) * 128,
                                          eg * 2048:eg * 2048 + 1024],
                                ob[:, 0:2, :],
                            )
                    if not fine:
                        nc.sync.dma_start(
                            outp.ap()[tb * 128:(tb + 1) * 128,
                                      eg * 2048 + 1024:(eg + 1) * 2048],
                            ob[:, 2:4, :],
                        )

                # Batch b's attention units are interleaved with batch
                # b-1's output-projection units: the projection's dense
                # matmul stream fills the PE whenever attention waits on
                # the exp chain.
                for b in range(B):
                    fill = (
                        [(eg, tb) for eg in range(2)
                         for tb in range(8 * (b - 1), 8 * (b - 1) + 8)]
                        if b > 0 else []
                    )
                    ui = 0
                    for g in range(GQ):
                        for it in range(2):
                            p2_unit(b, g, it)
                            for eg, tb in fill[2 * ui:2 * ui + 2]:
                                p3_unit(eg, tb)
                            ui += 1
                tail_units = [
                    (eg, tb) for eg in range(2)
                    for tb in range(8 * (B - 1), 8 * (B - 1) + 8)
                ]
                for i, (eg, tb) in enumerate(tail_units):
                    p3_unit(eg, tb, fine=(i >= len(tail_units) - 2))

    nc.finalize()
    return nc


def _get_program(split_kv: bool):
    if split_kv not in _PROG_CACHE:
        _PROG_CACHE[split_kv] = _build_program(split_kv)
    return _PROG_CACHE[split_kv]


def kernel(
    hidden_states, wq, wk, wv, wo, kv_cache, position_ids,
    kv_page_indices, kv_page_indptr, kv_last_page_lens, qo_indptr,
    _run_kwargs: dict | None = None,
):
    hidden_states = np.asarray(hidden_states, np.float32)
    wq = np.asarray(wq, np.float32)
    wk = np.asarray(wk, np.float32)
    wv = np.asarray(wv, np.float32)
    wo = np.asarray(wo, np.float32)
    position_ids = np.asarray(position_ids, np.int32)
    qo_indptr = np.asarray(qo_indptr, np.int64)

    nnz = hidden_states.shape[0]
    b = qo_indptr.shape[0] - 1
    assert nnz == T and b == B, (nnz, b)
    assert np.array_equal(qo_indptr, np.arange(B + 1, dtype=np.int64) * S), (
        "kernel assumes uniform sequence lengths of 1024"
    )

    # Page-gather order: the reference gathers pages in list order, so the
    # token with position p within its sequence lands at page-order rank p.
    # KV must be fed in rank order; the q path stays in token order.
    perm = np.empty(T, np.int64)
    identity = True
    for bi in range(B):
        pos_b = position_ids[bi * S:(bi + 1) * S].astype(np.int64)
        assert np.array_equal(np.sort(pos_b), np.arange(S)), (
            "kernel assumes positions cover 0..S-1 exactly once per sequence"
        )
        inv = np.empty(S, np.int64)
        inv[pos_b] = np.arange(S)
        perm[bi * S:(bi + 1) * S] = bi * S + inv
        if not np.array_equal(inv, np.arange(S)):
            identity = False

    hT16 = np.ascontiguousarray(hidden_states.T.astype(NP_BF16))
    coskv, sinkv = _rope_tables(np.arange(S, dtype=np.int64))
    tri = np.ascontiguousarray(
        (np.arange(128)[:, None] <= np.arange(128)[None, :]).astype(NP_BF16)
    )
    ones = np.ones((128, 128), NP_BF16)
    eye = np.eye(128, dtype=np.float32).astype(NP_BF16)

    split_kv = not identity
    nc = _get_program(split_kv)

    in_maps = []
    for c in range(NCORES):
        im = {
            "hT": hT16,
            "wqT": np.ascontiguousarray(wq[c * DQ:(c + 1) * DQ, :].T.astype(NP_BF16)),
            "wkT": np.ascontiguousarray(wk[c * HD:(c + 1) * HD, :].T.astype(NP_BF16)),
            "wvT": np.ascontiguousarray(wv[c * HD:(c + 1) * HD, :].T.astype(NP_BF16)),
            "woT": np.ascontiguousarray(wo[:, c * DQ:(c + 1) * DQ].T.astype(NP_BF16)),
            "coskv": coskv,
            "sinkv": sinkv,
            "trid": tri,
            "onesd": ones,
            "identd": eye,
        }
        if split_kv:
            im["hTkv"] = np.ascontiguousarray(hT16[:, perm])
            cosq, sinq = _rope_tables(position_ids)
            im["cosq"] = cosq
            im["sinq"] = sinq
        in_maps.append(im)

    res = run_bass_kernel_spmd(
        nc, in_maps, core_ids=list(range(NCORES)), **(_run_kwargs or {})
    )
    out = np.zeros((T, HID), np.float32)
    for c in range(NCORES):
        out += res.results[c]["outp"].astype(np.float32)
    kernel.last_results = res  # type: ignore[attr-defined]
    return out
